# revision 1
# baseline (speedup 1.0000x reference)
"""AnomalyNet (3-hop Chebyshev-style GNN) on 8 Trainium2 NeuronCores.

Strategy:
  - Shard nodes (rows) across 8 cores: core m owns rows [m*SHARD, (m+1)*SHARD).
  - Dense parts (x @ W_in, epilogue @ W_out) on TensorE, bf16.
  - Each spmm hop: the full feature table [N, 128] bf16 lives in HBM
    (AllGather of per-core shard outputs).  Each core gathers its edges'
    source rows with dma_gather (one 256B descriptor per edge), and reduces
    into rows via PE "segment matmuls": for each chunk of 128 edge slots, a
    host-built scatter matrix S [128 slots, 32 window-rows] (vals baked in,
    bf16) is the stationary operand; out[window, :] += S^T @ gathered.
  - int16 gather indices cap the table height at 32768, so gathers read one
    of two overlapping views of the table (rows [0, 32768) / [N-32768, N));
    edges in the overlap are assigned to balance chunk counts.
  - The chunk grid (windows x chunk counts) is shared by all 8 cores (SPMD:
    one instruction stream); per-core edge data (indices, S values) comes in
    via per-core input blobs.  Pad slots point at a real row with val 0.
  - 4th sparse pass (high_delta = L @ dn) reuses the same chunks/indices with
    a table of dn broadcast across 128 channels.
  - Global min/max of high_delta via a 2-element AllReduce(max) of
    [max, -min].
"""

import math
import os

import numpy as np
import ml_dtypes

import concourse.bacc as bacc
import concourse.bass as bass
import concourse.mybir as mybir
import concourse.tile as tile

F32 = mybir.dt.float32
BF16 = mybir.dt.bfloat16
I16 = mybir.dt.int16
AF = mybir.ActivationFunctionType
ALU = mybir.AluOpType
AXL = mybir.AxisListType

PCORES = 8
PART = 128  # partitions


class Cfg:
    def __init__(self, N=50000, E=800000, CIN=256, COUT=128, NCLS=7,
                 W=32, NBB=4, IDX_CAP=32768):
        assert N % PCORES == 0
        self.N, self.E, self.CIN, self.COUT, self.NCLS = N, E, CIN, COUT, NCLS
        self.SHARD = N // PCORES
        self.W = W                      # window rows per chunk
        self.WPB = PART // W            # windows per 128-row block
        self.NT = math.ceil(self.SHARD / PART)   # 128-row blocks per core
        self.LT = self.SHARD - (self.NT - 1) * PART  # rows in last block
        self.NW = math.ceil(self.SHARD / W)      # windows per core
        self.NBB = NBB                  # blocks per gather batch
        self.TA_H = min(N, IDX_CAP)     # table A = rows [0, TA_H)
        self.B_BASE = max(0, N - IDX_CAP)  # table B = rows [B_BASE, N)
        self.CINT = math.ceil(CIN / PART)  # 128-col tiles of CIN
        self.NTP = math.ceil(self.SHARD / 16) * 16  # shard rows padded (xbar)


# ---------------------------------------------------------------------------
# host-side preprocessing: build the (core-uniform) chunk grid and per-core
# index / S blobs
# ---------------------------------------------------------------------------

class Grid:
    """Uniform structure shared by all cores + per-core data blobs."""
    pass


def build_grid(cfg: Cfg, rows: np.ndarray, cols: np.ndarray, vals: np.ndarray):
    N, W = cfg.N, cfg.W
    SHARD, NW = cfg.SHARD, cfg.NW

    core_of = rows // SHARD
    rloc = rows - core_of * SHARD
    wid = rloc // W

    # category: 0 = A-only (c < B_BASE), 1 = flex, 2 = B-only (c >= TA_H)
    cat = np.ones(cfg.E, np.int8)
    cat[cols < cfg.B_BASE] = 0
    cat[cols >= cfg.TA_H] = 2

    order = np.lexsort((cat, wid, core_of))
    counts = np.zeros((PCORES, NW, 3), np.int64)
    np.add.at(counts, (core_of, wid, cat), 1)
    starts = np.zeros((PCORES, NW, 3), np.int64)
    starts.reshape(-1)[1:] = np.cumsum(counts.reshape(-1))[:-1]

    nA0 = counts[:, :, 0]
    nF = counts[:, :, 1]
    nB0 = counts[:, :, 2]
    ntot = nA0 + nF + nB0

    # uniform chunk counts per window
    kA = np.ceil(nA0.max(0) / 128).astype(int)
    kB = np.ceil(nB0.max(0) / 128).astype(int)
    need = np.ceil(ntot.max(0) / 128).astype(int)
    for w in range(NW):
        while kA[w] + kB[w] < max(need[w], 1):
            slackA = kA[w] * 128 - nA0[:, w].max()
            slackB = kB[w] * 128 - nB0[:, w].max()
            if kB[w] == 0 or slackA <= slackB:
                kA[w] += 1
            else:
                kB[w] += 1
    assert (kA[None, :] * 128 >= nA0).all()
    assert (kB[None, :] * 128 >= nB0).all()
    assert ((kA + kB)[None, :] * 128 >= ntot).all()

    g = Grid()
    g.kA, g.kB = kA, kB
    g.nchunks = int((kA + kB).sum())

    # chunk gidx base per window (A-chunks then B-chunks, windows in order)
    wbaseA = np.zeros(NW, np.int64)
    wbaseB = np.zeros(NW, np.int64)
    base = 0
    for w in range(NW):
        wbaseA[w] = base
        wbaseB[w] = base + kA[w]
        base += kA[w] + kB[w]

    # batches of NBB blocks; within a batch: chunks ordered by block, window,
    # A-chunks then B-chunks.  Gather call A covers the batch's A-chunks in
    # that order; call B likewise.
    nbatch = math.ceil(cfg.NT / cfg.NBB)
    batches = []
    idx_cols_total = 0
    for bi in range(nbatch):
        blocks = range(bi * cfg.NBB, min((bi + 1) * cfg.NBB, cfg.NT))
        ch = []
        for b in blocks:
            for w in range(b * cfg.WPB, min((b + 1) * cfg.WPB, NW)):
                nw_ch = int(kA[w] + kB[w])
                for i in range(nw_ch):
                    gi = int((wbaseA[w] if i < kA[w] else wbaseB[w] - kA[w])
                             + i)
                    ch.append(dict(g=gi, w=w, tab=0 if i < kA[w] else 1, b=b,
                                   first=(i == 0), last=(i == nw_ch - 1)))
        ncA = sum(1 for c in ch if c["tab"] == 0)
        ncB = len(ch) - ncA
        pA = pB = 0
        for c in ch:
            if c["tab"] == 0:
                c["pos"] = pA
                pA += 1
            else:
                c["pos"] = pB
                pB += 1
        batches.append(dict(blocks=list(blocks), ncA=ncA, ncB=ncB, chunks=ch,
                            idx_colA=idx_cols_total,
                            idx_colB=idx_cols_total + ncA * 8))
        idx_cols_total += (ncA + ncB) * 8
    g.batches = batches
    g.idx_cols_total = idx_cols_total

    # ---- per-core blobs
    g.sblobs = []
    g.iblobs = []
    for m in range(PCORES):
        S = np.zeros((g.nchunks, PART, W), np.float32)
        idx_all = np.zeros((g.nchunks, PART), np.int16)

        for w in range(NW):
            e0a, n0a = starts[m, w, 0], counts[m, w, 0]
            e0f, n0f = starts[m, w, 1], counts[m, w, 1]
            e0b, n0b = starts[m, w, 2], counts[m, w, 2]
            capA = kA[w] * 128
            fA = min(n0f, capA - n0a)
            assert n0b + (n0f - fA) <= kB[w] * 128

            selA = np.concatenate([order[e0a:e0a + n0a], order[e0f:e0f + fA]])
            selB = np.concatenate([order[e0f + fA:e0f + n0f],
                                   order[e0b:e0b + n0b]])
            for sel, k, gbase, off in (
                (selA, int(kA[w]), int(wbaseA[w]), 0),
                (selB, int(kB[w]), int(wbaseB[w]), cfg.B_BASE),
            ):
                ns = len(sel)
                if k == 0:
                    assert ns == 0
                    continue
                ids = (cols[sel] - off).astype(np.int16)
                vs = vals[sel]
                rr = (rloc[sel] - w * W).astype(np.int64)
                cap = k * 128
                pad_idx = ids[-1] if ns else np.int16(0)
                full_ids = np.full(cap, pad_idx, np.int16)
                full_ids[:ns] = ids
                for j in range(k):
                    gi = gbase + j
                    seg = np.arange(j * 128, min((j + 1) * 128, ns))
                    if len(seg):
                        S[gi, seg - j * 128, rr[seg]] = vs[seg]
                    idx_all[gi] = full_ids[j * 128:(j + 1) * 128]

        iblob = np.zeros((16, idx_cols_total), np.int16)
        for bt in batches:
            for c in bt["chunks"]:
                col0 = (bt["idx_colA"] if c["tab"] == 0 else bt["idx_colB"]) \
                    + c["pos"] * 8
                iblob[:, col0:col0 + 8] = idx_all[c["g"]].reshape(8, 16).T
        g.sblobs.append(S.astype(ml_dtypes.bfloat16))
        g.iblobs.append(iblob)
    return g


# ---------------------------------------------------------------------------
# kernel builder (SPMD graph, shared by all cores)
# ---------------------------------------------------------------------------

def _finalize(nc):
    nc.compile()


# Bacc.compile()'s late passes (library/act-table loads, hostgen rebases) run
# after its last generate_event_semaphores() and can leave DMA instructions
# with 2 sync waits; walrus's DIRECT2D lowering has a single wait slot and
# dies with "Too many sync wait commands".  Splice one more splitter pass in
# right before ISA codegen (the last point where inserted EventSemaphores
# still go through codegen).
import concourse.bacc as _bacc_mod

if not getattr(_bacc_mod.Bacc, "_evsem_patch", False):
    _orig_codegen = _bacc_mod.Bacc.codegen_inst_isa_subclasses

    def _codegen_with_split(self):
        self.generate_event_semaphores()
        return _orig_codegen(self)

    _bacc_mod.Bacc.codegen_inst_isa_subclasses = _codegen_with_split
    _bacc_mod.Bacc._evsem_patch = True


def build_nc(cfg: Cfg, g, debug=False):
    STAGE = int(os.environ.get("KSTAGE", "99"))
    BIGRING = os.environ.get("KBIGRING", "0") == "1"
    nc = bacc.Bacc("TRN2", target_bir_lowering=False, debug=debug,
                   num_devices=PCORES,
                   dynamic_dma_scratch_size=32768 if BIGRING else 16384)
    N, COUT, NCLS, W = cfg.N, cfg.COUT, cfg.NCLS, cfg.W
    SHARD, NT, LT = cfg.SHARD, cfg.NT, cfg.LT
    RG = [list(range(PCORES))]

    # ---------------- dram parameters
    xs = nc.declare_dram_parameter("xs", [cfg.CIN, SHARD], F32, isOutput=False)
    Wi = nc.declare_dram_parameter("Wi", [cfg.CIN, COUT], F32, isOutput=False)
    bi = nc.declare_dram_parameter("bi", [1, COUT], F32, isOutput=False)
    Wo = nc.declare_dram_parameter("Wo", [COUT, NCLS], F32, isOutput=False)
    bo = nc.declare_dram_parameter("bo", [1, NCLS], F32, isOutput=False)
    dl = nc.declare_dram_parameter("dl", [1, 1], F32, isOutput=False)
    av = nc.declare_dram_parameter("av", [1, 1], F32, isOutput=False)
    idt = nc.declare_dram_parameter("idt", [PART, PART], BF16, isOutput=False)
    sbl = nc.declare_dram_parameter("sblob", [g.nchunks, PART, W], BF16,
                                    isOutput=False)
    ibl = nc.declare_dram_parameter("iblob", [16, g.idx_cols_total], I16,
                                    isOutput=False)
    out = nc.declare_dram_parameter("out", [SHARD, NCLS], F32, isOutput=True)

    # ---------------- internal dram
    agin = [nc.dram_tensor(f"agin{k}", [SHARD, COUT], BF16) for k in range(4)]
    tbl = [nc.dram_tensor(f"tbl{k}", [N, COUT], BF16, addr_space="Shared")
           for k in range(4)]
    mm_in = nc.dram_tensor("mm_in", [1, 2], F32)
    mm_out = nc.dram_tensor("mm_out", [1, 2], F32, addr_space="Shared")

    def store_shard(dram, sb3):
        """sb3 [128, NT, COUT] -> dram [SHARD, COUT], row = t*128 + p."""
        if NT > 1:
            nc.sync.dma_start(
                dram[0:(NT - 1) * PART, :].rearrange("(t p) c -> p t c",
                                                     p=PART),
                sb3[:, 0:NT - 1, :])
        nc.sync.dma_start(dram[(NT - 1) * PART:SHARD, :], sb3[0:LT, NT - 1, :])

    def transpose(out_ap, in_ap, ident, k):
        nc.tensor.matmul(out_ap, in_ap, ident[0:k, 0:k], is_transpose=True,
                         start=True, stop=True, skip_group_check=True)

    with tile.TileContext(nc) as tc:
        with (
            tc.tile_pool(name="const", bufs=1) as cp,
            tc.tile_pool(name="resid", bufs=1) as rp,
            tc.tile_pool(name="gat", bufs=3) as gp,
            tc.tile_pool(name="stage", bufs=3) as sp,
            tc.tile_pool(name="pbig", bufs=4, space="PSUM") as pb,
            tc.tile_pool(name="psmall", bufs=2, space="PSUM") as ps,
            tc.tile_pool(name="phd", bufs=2, space="PSUM") as ph,
        ):
            # ---------------- constants
            Wi_sb = cp.tile([PART, cfg.CINT, COUT], BF16)
            for h in range(cfg.CINT):
                hi = min(cfg.CIN - h * PART, PART)
                nc.gpsimd.dma_start(Wi_sb[0:hi, h, :],
                                    Wi[h * PART:h * PART + hi, :])
            bi_sb = cp.tile([1, COUT], BF16)
            nc.gpsimd.dma_start(bi_sb[:], bi[:])
            Wo_sb = cp.tile([PART, NCLS], BF16)
            nc.gpsimd.dma_start(Wo_sb[0:COUT, :], Wo[:])
            bo_sb = cp.tile([1, NCLS], BF16)
            nc.gpsimd.dma_start(bo_sb[:], bo[:])
            id_bf = cp.tile([PART, PART], BF16)
            nc.sync.dma_start(id_bf[:], idt[:])
            id_f32 = cp.tile([PART, PART], F32)
            nc.vector.tensor_copy(id_f32[:], id_bf[:])
            ones1b = cp.tile([1, PART], BF16)
            nc.vector.memset(ones1b[:], 1.0)
            ones1f = cp.tile([1, PART], F32)
            nc.vector.memset(ones1f[:], 1.0)
            onesCb = cp.tile([PART, COUT], BF16)
            nc.vector.memset(onesCb[:], 1.0)
            dl_sb = cp.tile([1, 1], F32)
            nc.sync.dma_start(dl_sb[:], dl[:])
            av_sb = cp.tile([1, 1], F32)
            nc.sync.dma_start(av_sb[:], av[:])

            idx_sb = cp.tile([PART, g.idx_cols_total], I16)
            for k in range(8):
                nc.sync.dma_start(idx_sb[16 * k:16 * (k + 1), :], ibl[:])
            s_sb = cp.tile([PART, g.nchunks, W], BF16)
            nc.sync.dma_start(s_sb[:], sbl.ap().rearrange("c p w -> p c w"))

            # resident feature tiles [128, NT, COUT]
            h_sb = rp.tile([PART, NT, COUT], BF16)
            t1_sb = rp.tile([PART, NT, COUT], BF16)
            t2_sb = rp.tile([PART, NT, COUT], BF16)
            u_sb = rp.tile([PART, NT, COUT], BF16)
            db_sb = rp.tile([PART, NT, COUT], BF16)
            dn_sb = rp.tile([PART, NT], F32)
            nc.vector.memset(dn_sb[:], 0.0)
            hd_sb = rp.tile([PART, NT], F32)
            ns_sb = rp.tile([PART, NT], F32)

            # ---------------- coefficients (f32 [1,1] lane ops)
            cofp = cp.tile([1, 8], F32)  # c2L c1L c0L1 n2d d2
            nc.vector.memset(cofp[:], 0.0)
            t_ = cp.tile([1, 6], F32)
            d_, a_ = dl_sb[:], av_sb[:]
            d2_ = t_[:, 0:1]
            nc.vector.tensor_mul(d2_, d_, d_)
            d3_ = t_[:, 1:2]
            nc.vector.tensor_mul(d3_, d2_, d_)
            da_ = t_[:, 2:3]
            nc.vector.tensor_mul(da_, d_, a_)
            d2a_ = t_[:, 3:4]
            nc.vector.tensor_mul(d2a_, d2_, a_)
            nc.vector.scalar_tensor_tensor(cofp[:, 0:1], d_, -3.0, a_,
                                           ALU.mult, ALU.subtract)  # -3d - a
            t30 = t_[:, 4:5]
            nc.vector.tensor_scalar_mul(t30, d2_, 3.0)
            nc.vector.scalar_tensor_tensor(cofp[:, 1:2], da_, 2.0, t30,
                                           ALU.mult, ALU.add)  # 3d2 + 2da
            t31 = t_[:, 5:6]
            nc.vector.tensor_add(t31, d3_, d2a_)
            nc.vector.tensor_scalar(cofp[:, 2:3], t31, -1.0, -1.0,
                                    ALU.mult, ALU.add)  # -(d3 + d2a) - 1
            nc.vector.tensor_scalar_mul(cofp[:, 3:4], d_, -2.0)  # -2d
            nc.vector.tensor_copy(cofp[:, 4:5], d2_)  # d2
            pco = ps.tile([PART, 8], F32, tag="sm")
            nc.tensor.matmul(pco[:], ones1f[:], cofp[:], start=True, stop=True)
            cf_sb = cp.tile([PART, 8], F32)
            nc.scalar.copy(cf_sb[:], pco[:])
            C2L, C1L, C0L1 = cf_sb[:, 0:1], cf_sb[:, 1:2], cf_sb[:, 2:3]
            N2D, D2C = cf_sb[:, 3:4], cf_sb[:, 4:5]

            # ---------------- prologue: h = tanh(x @ Wi + bi)
            with tc.tile_pool(name="xt", bufs=1 if not BIGRING else 3) as xp:
                xT = None
                if not BIGRING:
                    xT = xp.tile([PART, cfg.CINT, SHARD], BF16)
                    for h in range(cfg.CINT):
                        hi = min(cfg.CIN - h * PART, PART)
                        nc.gpsimd.dma_start(xT[0:hi, h, :],
                                            xs[h * PART:h * PART + hi, :])
                for t in range(NT):
                    vp = LT if t == NT - 1 else PART
                    r0 = t * PART
                    if BIGRING:
                        xTt = xp.tile([PART, cfg.CINT, PART], BF16,
                                      name="xTt", tag="xTt")
                        for h in range(cfg.CINT):
                            hi = min(cfg.CIN - h * PART, PART)
                            nc.gpsimd.dma_start(
                                xTt[0:hi, h, 0:vp],
                                xs[h * PART:h * PART + hi, r0:r0 + vp])
                    pt = pb.tile([PART, COUT], F32, tag="big")
                    for h in range(cfg.CINT):
                        lhs = (xT[:, h, r0:r0 + vp] if not BIGRING
                               else xTt[:, h, 0:vp])
                        nc.tensor.matmul(pt[0:vp, :], lhs,
                                         Wi_sb[:, h, :], start=(h == 0),
                                         stop=False)
                    nc.tensor.matmul(pt[0:vp, :], ones1b[:, 0:vp], bi_sb[:],
                                     start=False, stop=True)
                    nc.scalar.activation(h_sb[0:vp, t, :], pt[0:vp, :],
                                         AF.Tanh)
            store_shard(agin[0], h_sb)
            nc.gpsimd.collective_compute("AllGather", ALU.bypass, RG,
                                         ins=[agin[0][:]], outs=[tbl[0][:]])

            def _early_out():
                for t in range(NT):
                    vp = LT if t == NT - 1 else PART
                    zt = sp.tile([PART, NCLS], F32, tag="es", name="zt")
                    nc.vector.memset(zt[:], 0.0)
                    nc.gpsimd.dma_start(out[t * PART:t * PART + vp, :],
                                        zt[0:vp, :])

            # ---------------- generic sparse hop
            def hop(tbl_in, nfree, on_block_done):
                tA = tbl_in[0:cfg.TA_H, :]
                tB = tbl_in[cfg.B_BASE:N, :]
                for bt in g.batches:
                    gA = gB = None
                    if bt["ncA"]:
                        gA = gp.tile([PART, bt["ncA"], COUT], BF16, tag="g")
                        nc.gpsimd.dma_gather(
                            gA[:], tA,
                            idx_sb[:, bt["idx_colA"]:
                                   bt["idx_colA"] + bt["ncA"] * 8],
                            bt["ncA"] * PART, bt["ncA"] * PART, COUT,
                            single_packet=False)
                    if bt["ncB"]:
                        gB = gp.tile([PART, bt["ncB"], COUT], BF16, tag="g")
                        nc.gpsimd.dma_gather(
                            gB[:], tB,
                            idx_sb[:, bt["idx_colB"]:
                                   bt["idx_colB"] + bt["ncB"] * 8],
                            bt["ncB"] * PART, bt["ncB"] * PART, COUT,
                            single_packet=False)
                    cur_b = -1
                    psum = None
                    for c in bt["chunks"]:
                        if c["b"] != cur_b:
                            if cur_b >= 0:
                                on_block_done(cur_b, psum)
                            cur_b = c["b"]
                            if nfree == COUT:
                                psum = pb.tile([PART, COUT], F32, tag="big",
                                               name="psum_hop")
                            else:
                                psum = ph.tile([PART, 1], F32, tag="hd",
                                               name="psum_hd")
                        woff = (c["w"] % cfg.WPB) * W
                        gt = gA if c["tab"] == 0 else gB
                        nc.tensor.matmul(
                            psum[woff:woff + W, 0:nfree],
                            s_sb[:, c["g"], :],
                            gt[:, c["pos"], 0:nfree],
                            start=c["first"], stop=c["last"],
                            skip_group_check=True,
                            tile_position=(0, woff))
                    if cur_b >= 0:
                        on_block_done(cur_b, psum)

            if STAGE < 1:
                _early_out()
                _finalize(nc)
                return nc

            # hop 1: T1
            def done1(b, psum):
                vp = LT if b == NT - 1 else PART
                nc.scalar.copy(t1_sb[0:vp, b, :], psum[0:vp, :])
            hop(tbl[0], COUT, done1)
            store_shard(agin[1], t1_sb)
            nc.gpsimd.collective_compute("AllGather", ALU.bypass, RG,
                                         ins=[agin[1][:]], outs=[tbl[1][:]])

            if STAGE < 2:
                _early_out()
                _finalize(nc)
                return nc

            # hop 2: T2 (+ db = T2 - 2d*T1 + d2*h)
            def done2(b, psum):
                vp = LT if b == NT - 1 else PART
                nc.scalar.copy(t2_sb[0:vp, b, :], psum[0:vp, :])
                e1 = sp.tile([PART, COUT], F32, tag="e1")
                nc.vector.scalar_tensor_tensor(
                    e1[0:vp, :], t1_sb[0:vp, b, :], N2D[0:vp, :],
                    psum[0:vp, :], ALU.mult, ALU.add)
                nc.vector.scalar_tensor_tensor(
                    db_sb[0:vp, b, :], h_sb[0:vp, b, :], D2C[0:vp, :],
                    e1[0:vp, :], ALU.mult, ALU.add)
            hop(tbl[1], COUT, done2)
            store_shard(agin[2], t2_sb)
            nc.gpsimd.collective_compute("AllGather", ALU.bypass, RG,
                                         ins=[agin[2][:]], outs=[tbl[2][:]])

            if STAGE < 3:
                _early_out()
                _finalize(nc)
                return nc

            # hop 3: u = T3 + c2L*T2 + c1L*T1 + (c0L-1)*h ; dn2 = row sumsq
            def done3(b, psum):
                vp = LT if b == NT - 1 else PART
                e1 = sp.tile([PART, COUT], F32, tag="e1")
                nc.vector.scalar_tensor_tensor(
                    e1[0:vp, :], t2_sb[0:vp, b, :], C2L[0:vp, :],
                    psum[0:vp, :], ALU.mult, ALU.add)
                nc.vector.scalar_tensor_tensor(
                    e1[0:vp, :], t1_sb[0:vp, b, :], C1L[0:vp, :],
                    e1[0:vp, :], ALU.mult, ALU.add)
                nc.vector.scalar_tensor_tensor(
                    e1[0:vp, :], h_sb[0:vp, b, :], C0L1[0:vp, :],
                    e1[0:vp, :], ALU.mult, ALU.add)
                nc.vector.tensor_copy(u_sb[0:vp, b, :], e1[0:vp, :])
                sq = sp.tile([PART, COUT], F32, tag="sq")
                nc.scalar.activation(sq[0:vp, :], e1[0:vp, :], AF.Square,
                                     accum_out=dn_sb[0:vp, b:b + 1])
            hop(tbl[2], COUT, done3)

            if STAGE < 4:
                _early_out()
                _finalize(nc)
                return nc

            # dn = sqrt(dn2); dnb table rows = dn broadcast over channels
            nc.scalar.sqrt(dn_sb[:], dn_sb[:])
            for t in range(NT):
                vp = LT if t == NT - 1 else PART
                dnb_t = sp.tile([PART, COUT], BF16, tag="fr")
                nc.vector.tensor_scalar_mul(dnb_t[0:vp, :], onesCb[0:vp, :],
                                            dn_sb[0:vp, t:t + 1])
                nc.sync.dma_start(agin[3][t * PART:t * PART + vp, :],
                                  dnb_t[0:vp, :])
            nc.gpsimd.collective_compute("AllGather", ALU.bypass, RG,
                                         ins=[agin[3][:]], outs=[tbl[3][:]])

            if STAGE < 5:
                _early_out()
                _finalize(nc)
                return nc

            # hop 4: hd = L @ dn
            def done4(b, psum):
                vp = LT if b == NT - 1 else PART
                nc.scalar.copy(hd_sb[0:vp, b:b + 1], psum[0:vp, 0:1])
            hop(tbl[3], 1, done4)

            if STAGE < 6:
                _early_out()
                _finalize(nc)
                return nc

            # ---------------- global min/max -> AllReduce(max) of [max, -min]
            mx1 = sp.tile([PART, 1], F32, tag="mm")
            mn1 = sp.tile([PART, 1], F32, tag="mm")
            if NT > 1:
                nc.vector.reduce_max(mx1[:], hd_sb[:, 0:NT - 1], axis=AXL.X)
                nc.vector.tensor_reduce(mn1[:], hd_sb[:, 0:NT - 1], axis=AXL.X, op=ALU.min)
                nc.vector.tensor_tensor(mx1[0:LT, :], mx1[0:LT, :],
                                        hd_sb[0:LT, NT - 1:NT], op=ALU.max)
                nc.vector.tensor_tensor(mn1[0:LT, :], mn1[0:LT, :],
                                        hd_sb[0:LT, NT - 1:NT], op=ALU.min)
            else:
                nc.vector.reduce_max(mx1[0:LT, :], hd_sb[0:LT, :], axis=AXL.X)
                nc.vector.tensor_reduce(mn1[0:LT, :], hd_sb[0:LT, :],
                                        axis=AXL.X, op=ALU.min)
            nc.vector.tensor_scalar_mul(mn1[:], mn1[:], -1.0)
            pmx = ps.tile([PART, PART], F32, tag="sm")
            transpose(pmx[0:1, 0:PART], mx1[:], id_f32, PART)
            pmn = ps.tile([PART, PART], F32, tag="sm")
            transpose(pmn[0:1, 0:PART], mn1[:], id_f32, PART)
            mm_sb = sp.tile([1, 2], F32, tag="mm2")
            nc.vector.reduce_max(mm_sb[0:1, 0:1], pmx[0:1, 0:PART], axis=AXL.X)
            nc.vector.reduce_max(mm_sb[0:1, 1:2], pmn[0:1, 0:PART], axis=AXL.X)
            nc.sync.dma_start(mm_in[:], mm_sb[:])
            nc.gpsimd.collective_compute("AllReduce", ALU.max, RG,
                                         ins=[mm_in[:]], outs=[mm_out[:]])
            mmg = sp.tile([1, 2], F32, tag="mm2")
            nc.sync.dma_start(mmg[:], mm_out[:])

            # s = 2a/(mx - mn); ns = (hd + (-mn)) * s  (= normal * 2a)
            sc = cp.tile([1, 2], F32)
            nc.vector.tensor_add(sc[:, 0:1], mmg[:, 0:1], mmg[:, 1:2])
            nc.vector.reciprocal(sc[:, 0:1], sc[:, 0:1])
            nc.vector.tensor_mul(sc[:, 0:1], sc[:, 0:1], av_sb[:])
            nc.vector.tensor_scalar_mul(sc[:, 0:1], sc[:, 0:1], 2.0)
            nc.vector.tensor_copy(sc[:, 1:2], mmg[:, 1:2])
            pbc = ps.tile([PART, PART], F32, tag="sm")
            nc.tensor.matmul(pbc[0:PART, 0:2], ones1f[:], sc[:],
                             start=True, stop=True, skip_group_check=True)
            bc = cp.tile([PART, 2], F32)
            nc.scalar.copy(bc[:], pbc[0:PART, 0:2])
            nc.vector.tensor_scalar(ns_sb[:], hd_sb[:], bc[:, 1:2],
                                    bc[:, 0:1], ALU.add, ALU.mult)

            if STAGE < 7:
                _early_out()
                _finalize(nc)
                return nc

            # ---------------- epilogue
            for t in range(NT):
                vp = LT if t == NT - 1 else PART
                f1 = sp.tile([PART, COUT], F32, tag="e1")
                nc.vector.scalar_tensor_tensor(
                    f1[0:vp, :], db_sb[0:vp, t, :], ns_sb[0:vp, t:t + 1],
                    u_sb[0:vp, t, :], ALU.mult, ALU.add)
                nc.vector.tensor_add(f1[0:vp, :], f1[0:vp, :],
                                     h_sb[0:vp, t, :])
                fr = sp.tile([PART, COUT], BF16, tag="fr")
                nc.scalar.activation(fr[0:vp, :], f1[0:vp, :], AF.Relu)
                ptr = pb.tile([PART, PART], BF16, tag="big")
                transpose(ptr[0:COUT, 0:vp], fr[0:vp, :], id_bf, vp)
                frT = sp.tile([PART, PART], BF16, tag="frT")
                nc.scalar.copy(frT[0:COUT, 0:vp], ptr[0:COUT, 0:vp])
                py = ps.tile([PART, NCLS], F32, tag="sm")
                nc.tensor.matmul(py[0:vp, :], frT[:, 0:vp], Wo_sb[:],
                                 start=True, stop=False)
                nc.tensor.matmul(py[0:vp, :], ones1b[:, 0:vp], bo_sb[:],
                                 start=False, stop=True)
                nm = sp.tile([PART, 1], F32, tag="nm")
                nc.vector.reduce_max(nm[0:vp, :], py[0:vp, :], axis=AXL.X, negate=True)
                es = sp.tile([PART, NCLS], F32, tag="es")
                ssum = sp.tile([PART, 1], F32, tag="nm")
                nc.scalar.activation(es[0:vp, :], py[0:vp, :], AF.Exp,
                                     bias=nm[0:vp, :], accum_out=ssum[0:vp, :])
                lse = sp.tile([PART, 1], F32, tag="nm")
                nc.scalar.activation(lse[0:vp, :], ssum[0:vp, :], AF.Ln)
                ot = sp.tile([PART, NCLS], F32, tag="es")
                nc.vector.tensor_scalar(ot[0:vp, :], py[0:vp, :],
                                        nm[0:vp, :], lse[0:vp, :],
                                        ALU.add, ALU.subtract)
                r0 = t * PART
                nc.sync.dma_start(out[r0:r0 + vp, :], ot[0:vp, :])

    _finalize(nc)
    return nc


# ---------------------------------------------------------------------------
# entry point
# ---------------------------------------------------------------------------

def _in_maps(cfg, g, x, W_in, b_in, W_out, b_out, delta, a):
    ident = np.eye(PART, dtype=np.float32).astype(ml_dtypes.bfloat16)
    maps = []
    for m in range(PCORES):
        maps.append({
            "xs": np.ascontiguousarray(
                x[m * cfg.SHARD:(m + 1) * cfg.SHARD].T).astype(np.float32),
            "Wi": np.ascontiguousarray(W_in).astype(np.float32),
            "bi": b_in.reshape(1, -1).astype(np.float32),
            "Wo": np.ascontiguousarray(W_out).astype(np.float32),
            "bo": b_out.reshape(1, -1).astype(np.float32),
            "dl": delta.reshape(1, 1).astype(np.float32),
            "av": a.reshape(1, 1).astype(np.float32),
            "idt": ident,
            "sblob": g.sblobs[m],
            "iblob": g.iblobs[m],
        })
    return maps


def prepare(x, vals, W_in, b_in, delta, a, W_out, b_out, rows, cols,
            debug=False, **cfg_kw):
    x = np.asarray(x)
    cfg = Cfg(N=x.shape[0], E=len(np.asarray(vals)), CIN=x.shape[1],
              COUT=np.asarray(W_in).shape[1], NCLS=np.asarray(W_out).shape[1],
              **cfg_kw)
    g = build_grid(cfg, np.asarray(rows), np.asarray(cols),
                   np.asarray(vals, np.float32))
    nc = build_nc(cfg, g, debug=debug)
    maps = _in_maps(cfg, g, x, np.asarray(W_in), np.asarray(b_in),
                    np.asarray(W_out), np.asarray(b_out),
                    np.asarray(delta), np.asarray(a))
    return cfg, g, nc, maps


def kernel(x, vals, W_in, b_in, delta, a, W_out, b_out, rows, cols):
    from concourse.bass_utils import run_bass_kernel_spmd

    cfg, g, nc, maps = prepare(x, vals, W_in, b_in, delta, a, W_out, b_out,
                               rows, cols)
    res = run_bass_kernel_spmd(nc, maps, core_ids=list(range(PCORES)))
    return np.concatenate([res.results[m]["out"] for m in range(PCORES)], 0)



# revision 14
# speedup vs baseline: 2.1780x; 2.1780x over previous
"""AnomalyNet (3-hop Chebyshev-style GNN) on 8 Trainium2 NeuronCores.

Strategy:
  - Shard nodes (rows) across 8 cores: core m owns rows [m*SHARD, (m+1)*SHARD).
  - Dense parts (x @ W_in, epilogue @ W_out) on TensorE, bf16.
  - Each spmm hop: the full feature table [N, 128] bf16 lives in HBM
    (AllGather of per-core shard outputs).  Each core gathers its edges'
    source rows with dma_gather (one 256B descriptor per edge), and reduces
    into rows via PE "segment matmuls": for each chunk of 128 edge slots, a
    host-built scatter matrix S [128 slots, 32 window-rows] (vals baked in,
    bf16) is the stationary operand; out[window, :] += S^T @ gathered.
  - int16 gather indices cap the table height at 32768, so gathers read one
    of two overlapping views of the table (rows [0, 32768) / [N-32768, N));
    edges in the overlap are assigned to balance chunk counts.
  - The chunk grid (windows x chunk counts) is shared by all 8 cores (SPMD:
    one instruction stream); per-core edge data (indices, S values) comes in
    via per-core input blobs.  Pad slots point at a real row with val 0.
  - 4th sparse pass (high_delta = L @ dn) reuses the same chunks/indices with
    a table of dn broadcast across 128 channels.
  - Global min/max of high_delta via a 2-element AllReduce(max) of
    [max, -min].
"""

import math
import os

import numpy as np
import ml_dtypes

import concourse.bacc as bacc
import concourse.bass as bass
import concourse.mybir as mybir
import concourse.tile as tile

F32 = mybir.dt.float32
BF16 = mybir.dt.bfloat16
I16 = mybir.dt.int16
AF = mybir.ActivationFunctionType
ALU = mybir.AluOpType
AXL = mybir.AxisListType

PCORES = 8
PART = 128  # partitions


class Cfg:
    def __init__(self, N=50000, E=800000, CIN=256, COUT=128, NCLS=7,
                 W=32, NBB=2, IDX_CAP=32768):
        assert N % PCORES == 0
        self.N, self.E, self.CIN, self.COUT, self.NCLS = N, E, CIN, COUT, NCLS
        self.SHARD = N // PCORES
        self.W = W                      # window rows per chunk
        self.WPB = PART // W            # windows per 128-row block
        self.NT = math.ceil(self.SHARD / PART)   # 128-row blocks per core
        self.LT = self.SHARD - (self.NT - 1) * PART  # rows in last block
        self.NW = math.ceil(self.SHARD / W)      # windows per core
        self.NBB = NBB                  # blocks per gather batch
        self.TA_H = min(N, IDX_CAP)     # table A = rows [0, TA_H)
        self.B_BASE = max(0, N - IDX_CAP)  # table B = rows [B_BASE, N)
        self.CINT = math.ceil(CIN / PART)  # 128-col tiles of CIN
        self.NTP = math.ceil(self.SHARD / 16) * 16  # shard rows padded (xbar)


# ---------------------------------------------------------------------------
# host-side preprocessing: build the (core-uniform) chunk grid and per-core
# index / S blobs
# ---------------------------------------------------------------------------

class Grid:
    """Uniform structure shared by all cores + per-core data blobs."""
    pass


def build_grid(cfg: Cfg, rows: np.ndarray, cols: np.ndarray, vals: np.ndarray):
    N, W = cfg.N, cfg.W
    SHARD, NW = cfg.SHARD, cfg.NW

    core_of = rows // SHARD
    rloc = rows - core_of * SHARD
    wid = rloc // W

    # category: 0 = A-only (c < B_BASE), 1 = flex, 2 = B-only (c >= TA_H)
    cat = np.ones(cfg.E, np.int8)
    cat[cols < cfg.B_BASE] = 0
    cat[cols >= cfg.TA_H] = 2

    order = np.lexsort((cat, wid, core_of))
    counts = np.zeros((PCORES, NW, 3), np.int64)
    np.add.at(counts, (core_of, wid, cat), 1)
    starts = np.zeros((PCORES, NW, 3), np.int64)
    starts.reshape(-1)[1:] = np.cumsum(counts.reshape(-1))[:-1]

    nA0 = counts[:, :, 0]
    nF = counts[:, :, 1]
    nB0 = counts[:, :, 2]
    ntot = nA0 + nF + nB0

    # uniform chunk counts per window
    kA = np.ceil(nA0.max(0) / 128).astype(int)
    kB = np.ceil(nB0.max(0) / 128).astype(int)
    need = np.ceil(ntot.max(0) / 128).astype(int)
    for w in range(NW):
        while kA[w] + kB[w] < max(need[w], 1):
            slackA = kA[w] * 128 - nA0[:, w].max()
            slackB = kB[w] * 128 - nB0[:, w].max()
            if kB[w] == 0 or slackA <= slackB:
                kA[w] += 1
            else:
                kB[w] += 1
    assert (kA[None, :] * 128 >= nA0).all()
    assert (kB[None, :] * 128 >= nB0).all()
    assert ((kA + kB)[None, :] * 128 >= ntot).all()

    g = Grid()
    g.kA, g.kB = kA, kB
    g.nchunks = int((kA + kB).sum())

    # chunk gidx base per window (A-chunks then B-chunks, windows in order)
    wbaseA = np.zeros(NW, np.int64)
    wbaseB = np.zeros(NW, np.int64)
    base = 0
    for w in range(NW):
        wbaseA[w] = base
        wbaseB[w] = base + kA[w]
        base += kA[w] + kB[w]

    # batches of NBB blocks; within a batch: chunks ordered by block, window,
    # A-chunks then B-chunks.  Gather call A covers the batch's A-chunks in
    # that order; call B likewise.
    nbatch = math.ceil(cfg.NT / cfg.NBB)
    batches = []
    idx_cols_total = 0
    for bi in range(nbatch):
        blocks = range(bi * cfg.NBB, min((bi + 1) * cfg.NBB, cfg.NT))
        ch = []
        for b in blocks:
            for w in range(b * cfg.WPB, min((b + 1) * cfg.WPB, NW)):
                nw_ch = int(kA[w] + kB[w])
                for i in range(nw_ch):
                    gi = int((wbaseA[w] if i < kA[w] else wbaseB[w] - kA[w])
                             + i)
                    ch.append(dict(g=gi, w=w, tab=0 if i < kA[w] else 1, b=b,
                                   first=(i == 0), last=(i == nw_ch - 1)))
        ncA = sum(1 for c in ch if c["tab"] == 0)
        ncB = len(ch) - ncA
        pA = pB = 0
        for c in ch:
            if c["tab"] == 0:
                c["pos"] = pA
                pA += 1
            else:
                c["pos"] = pB
                pB += 1
        batches.append(dict(blocks=list(blocks), ncA=ncA, ncB=ncB, chunks=ch,
                            idx_colA=idx_cols_total,
                            idx_colB=idx_cols_total + ncA * 8))
        idx_cols_total += (ncA + ncB) * 8
    g.batches = batches
    g.idx_cols_total = idx_cols_total

    # ---- per-core blobs
    g.sblobs = []
    g.iblobs = []
    for m in range(PCORES):
        S = np.zeros((g.nchunks, PART, W), np.float32)
        idx_all = np.zeros((g.nchunks, PART), np.int16)

        for w in range(NW):
            e0a, n0a = starts[m, w, 0], counts[m, w, 0]
            e0f, n0f = starts[m, w, 1], counts[m, w, 1]
            e0b, n0b = starts[m, w, 2], counts[m, w, 2]
            capA = kA[w] * 128
            fA = min(n0f, capA - n0a)
            assert n0b + (n0f - fA) <= kB[w] * 128

            selA = np.concatenate([order[e0a:e0a + n0a], order[e0f:e0f + fA]])
            selB = np.concatenate([order[e0f + fA:e0f + n0f],
                                   order[e0b:e0b + n0b]])
            for sel, k, gbase, off in (
                (selA, int(kA[w]), int(wbaseA[w]), 0),
                (selB, int(kB[w]), int(wbaseB[w]), cfg.B_BASE),
            ):
                ns = len(sel)
                if k == 0:
                    assert ns == 0
                    continue
                ids = (cols[sel] - off).astype(np.int16)
                vs = vals[sel]
                rr = (rloc[sel] - w * W).astype(np.int64)
                cap = k * 128
                pad_idx = ids[-1] if ns else np.int16(0)
                full_ids = np.full(cap, pad_idx, np.int16)
                full_ids[:ns] = ids
                for j in range(k):
                    gi = gbase + j
                    seg = np.arange(j * 128, min((j + 1) * 128, ns))
                    if len(seg):
                        S[gi, seg - j * 128, rr[seg]] = vs[seg]
                    idx_all[gi] = full_ids[j * 128:(j + 1) * 128]

        iblob = np.zeros((16, idx_cols_total), np.int16)
        for bt in batches:
            for c in bt["chunks"]:
                col0 = (bt["idx_colA"] if c["tab"] == 0 else bt["idx_colB"]) \
                    + c["pos"] * 8
                iblob[:, col0:col0 + 8] = idx_all[c["g"]].reshape(8, 16).T
        g.sblobs.append(S.astype(ml_dtypes.bfloat16))
        g.iblobs.append(iblob)
    return g


# ---------------------------------------------------------------------------
# kernel builder (SPMD graph, shared by all cores)
# ---------------------------------------------------------------------------

def _finalize(nc):
    nc.compile()


# Bacc.compile()'s late passes (library/act-table loads, hostgen rebases) run
# after its last generate_event_semaphores() and can leave DMA instructions
# with 2 sync waits; walrus's DIRECT2D lowering has a single wait slot and
# dies with "Too many sync wait commands".  Splice one more splitter pass in
# right before ISA codegen (the last point where inserted EventSemaphores
# still go through codegen).
import concourse.bacc as _bacc_mod

if not getattr(_bacc_mod.Bacc, "_evsem_patch", False):
    _orig_codegen = _bacc_mod.Bacc.codegen_inst_isa_subclasses

    def _codegen_with_split(self):
        self.generate_event_semaphores()
        return _orig_codegen(self)

    _bacc_mod.Bacc.codegen_inst_isa_subclasses = _codegen_with_split
    _bacc_mod.Bacc._evsem_patch = True


def build_nc(cfg: Cfg, g, debug=False):
    STAGE = int(os.environ.get("KSTAGE", "99"))
    NQ = int(os.environ.get("KQ", "4"))  # SWDGE queues for gather desc-gen
    nc = bacc.Bacc("TRN2", target_bir_lowering=False, debug=debug,
                   num_devices=PCORES, num_swdge_queues=NQ,
                   dynamic_dma_scratch_size=16384)
    N, COUT, NCLS, W = cfg.N, cfg.COUT, cfg.NCLS, cfg.W
    SHARD, NT, LT = cfg.SHARD, cfg.NT, cfg.LT
    RG = [list(range(PCORES))]

    # ---------------- dram parameters
    xs = nc.declare_dram_parameter("xs", [cfg.CIN, SHARD], BF16, isOutput=False)
    Wi = nc.declare_dram_parameter("Wi", [cfg.CIN, COUT], F32, isOutput=False)
    bi = nc.declare_dram_parameter("bi", [1, COUT], F32, isOutput=False)
    Wo = nc.declare_dram_parameter("Wo", [COUT, NCLS], F32, isOutput=False)
    bo = nc.declare_dram_parameter("bo", [1, NCLS], F32, isOutput=False)
    dl = nc.declare_dram_parameter("dl", [1, 1], F32, isOutput=False)
    av = nc.declare_dram_parameter("av", [1, 1], F32, isOutput=False)
    idt = nc.declare_dram_parameter("idt", [PART, PART], BF16, isOutput=False)
    sbl = nc.declare_dram_parameter("sblob", [g.nchunks, PART, W], BF16,
                                    isOutput=False)
    ibl = nc.declare_dram_parameter("iblob", [16, g.idx_cols_total], I16,
                                    isOutput=False)
    out = nc.declare_dram_parameter("out", [SHARD, NCLS], F32, isOutput=True)

    # ---------------- internal dram
    agin = [nc.dram_tensor(f"agin{k}", [SHARD, COUT], BF16) for k in range(4)]
    tbl = [nc.dram_tensor(f"tbl{k}", [N, COUT], BF16, addr_space="Shared")
           for k in range(4)]
    mm_in = nc.dram_tensor("mm_in", [1, 2], F32)
    mm_out = nc.dram_tensor("mm_out", [1, 2], F32, addr_space="Shared")

    def store_shard(dram, sb3):
        """sb3 [128, NT, COUT] -> dram [SHARD, COUT], row = t*128 + p."""
        if NT > 1:
            nc.sync.dma_start(
                dram[0:(NT - 1) * PART, :].rearrange("(t p) c -> p t c",
                                                     p=PART),
                sb3[:, 0:NT - 1, :])
        nc.sync.dma_start(dram[(NT - 1) * PART:SHARD, :], sb3[0:LT, NT - 1, :])

    def transpose(out_ap, in_ap, ident, k):
        nc.tensor.matmul(out_ap, in_ap, ident[0:k, 0:k], is_transpose=True,
                         start=True, stop=True, skip_group_check=True)

    with tile.TileContext(nc) as tc:
        with (
            tc.tile_pool(name="const", bufs=1) as cp,
            tc.tile_pool(name="resid", bufs=1) as rp,
            tc.tile_pool(name="gat", bufs=2 * NQ) as gp,
            tc.tile_pool(name="stage", bufs=3) as sp,
            tc.tile_pool(name="pbig", bufs=4, space="PSUM") as pb,
            tc.tile_pool(name="psmall", bufs=2, space="PSUM") as ps,
            tc.tile_pool(name="phd", bufs=2, space="PSUM") as ph,
        ):
            # ---------------- constants
            Wi_sb = cp.tile([PART, cfg.CINT, COUT], BF16)
            for h in range(cfg.CINT):
                hi = min(cfg.CIN - h * PART, PART)
                nc.gpsimd.dma_start(Wi_sb[0:hi, h, :],
                                    Wi[h * PART:h * PART + hi, :])
            bi_sb = cp.tile([1, COUT], BF16)
            nc.gpsimd.dma_start(bi_sb[:], bi[:])
            Wo_sb = cp.tile([PART, NCLS], BF16)
            nc.gpsimd.dma_start(Wo_sb[0:COUT, :], Wo[:])
            bo_sb = cp.tile([1, NCLS], BF16)
            nc.gpsimd.dma_start(bo_sb[:], bo[:])
            id_bf = cp.tile([PART, PART], BF16)
            nc.sync.dma_start(id_bf[:], idt[:])
            id_f32 = cp.tile([PART, PART], F32)
            nc.vector.tensor_copy(id_f32[:], id_bf[:])
            ones1b = cp.tile([1, PART], BF16)
            nc.vector.memset(ones1b[:], 1.0)
            ones1f = cp.tile([1, PART], F32)
            nc.vector.memset(ones1f[:], 1.0)
            onesCb = cp.tile([PART, COUT], BF16)
            nc.vector.memset(onesCb[:], 1.0)
            dl_sb = cp.tile([1, 1], F32)
            nc.sync.dma_start(dl_sb[:], dl[:])
            av_sb = cp.tile([1, 1], F32)
            nc.sync.dma_start(av_sb[:], av[:])

            idx_sb = cp.tile([PART, g.idx_cols_total], I16)
            for k in range(8):
                nc.sync.dma_start(idx_sb[16 * k:16 * (k + 1), :], ibl[:])
            s_sb = cp.tile([PART, g.nchunks, W], BF16)
            nc.sync.dma_start(s_sb[:], sbl.ap().rearrange("c p w -> p c w"))

            # resident feature tiles [128, NT, COUT]
            h_sb = rp.tile([PART, NT, COUT], BF16)
            t1_sb = rp.tile([PART, NT, COUT], BF16)
            t2_sb = rp.tile([PART, NT, COUT], BF16)
            u_sb = rp.tile([PART, NT, COUT], BF16)
            dn_sb = rp.tile([PART, NT], F32)
            nc.vector.memset(dn_sb[:], 0.0)
            hd_sb = rp.tile([PART, NT], F32)
            ns_sb = rp.tile([PART, NT], F32)

            # ---------------- coefficients (f32 [1,1] lane ops)
            cofp = cp.tile([1, 8], F32)  # c2L c1L c0L1 n2d d2
            nc.vector.memset(cofp[:], 0.0)
            t_ = cp.tile([1, 6], F32)
            d_, a_ = dl_sb[:], av_sb[:]
            d2_ = t_[:, 0:1]
            nc.vector.tensor_mul(d2_, d_, d_)
            d3_ = t_[:, 1:2]
            nc.vector.tensor_mul(d3_, d2_, d_)
            da_ = t_[:, 2:3]
            nc.vector.tensor_mul(da_, d_, a_)
            d2a_ = t_[:, 3:4]
            nc.vector.tensor_mul(d2a_, d2_, a_)
            nc.vector.scalar_tensor_tensor(cofp[:, 0:1], d_, -3.0, a_,
                                           ALU.mult, ALU.subtract)  # -3d - a
            t30 = t_[:, 4:5]
            nc.vector.tensor_scalar_mul(t30, d2_, 3.0)
            nc.vector.scalar_tensor_tensor(cofp[:, 1:2], da_, 2.0, t30,
                                           ALU.mult, ALU.add)  # 3d2 + 2da
            t31 = t_[:, 5:6]
            nc.vector.tensor_add(t31, d3_, d2a_)
            nc.vector.tensor_scalar(cofp[:, 2:3], t31, -1.0, -1.0,
                                    ALU.mult, ALU.add)  # -(d3 + d2a) - 1
            nc.vector.tensor_scalar_mul(cofp[:, 3:4], d_, -2.0)  # -2d
            nc.vector.tensor_copy(cofp[:, 4:5], d2_)  # d2
            pco = ps.tile([PART, 8], F32, tag="sm")
            nc.tensor.matmul(pco[:], ones1f[:], cofp[:], start=True, stop=True)
            cf_sb = cp.tile([PART, 8], F32)
            nc.scalar.copy(cf_sb[:], pco[:])
            C2L, C1L, C0L1 = cf_sb[:, 0:1], cf_sb[:, 1:2], cf_sb[:, 2:3]
            N2D, D2C = cf_sb[:, 3:4], cf_sb[:, 4:5]

            # ---------------- prologue: h = tanh(x @ Wi + bi)
            with tc.tile_pool(name="xt", bufs=3) as xp:
                for t in range(NT):
                    vp = LT if t == NT - 1 else PART
                    r0 = t * PART
                    xTt = xp.tile([PART, cfg.CINT, PART], BF16,
                                  name="xTt", tag="xTt")
                    for h in range(cfg.CINT):
                        hi = min(cfg.CIN - h * PART, PART)
                        nc.sync.dma_start(
                            xTt[0:hi, h, 0:vp],
                            xs[h * PART:h * PART + hi, r0:r0 + vp])
                    pt = pb.tile([PART, COUT], F32, tag="big")
                    for h in range(cfg.CINT):
                        nc.tensor.matmul(pt[0:vp, :], xTt[:, h, 0:vp],
                                         Wi_sb[:, h, :], start=(h == 0),
                                         stop=False)
                    nc.tensor.matmul(pt[0:vp, :], ones1b[:, 0:vp], bi_sb[:],
                                     start=False, stop=True)
                    nc.scalar.activation(h_sb[0:vp, t, :], pt[0:vp, :],
                                         AF.Tanh)
            store_shard(agin[0], h_sb)
            nc.gpsimd.collective_compute("AllGather", ALU.bypass, RG,
                                         ins=[agin[0][:]], outs=[tbl[0][:]])

            def _early_out():
                for t in range(NT):
                    vp = LT if t == NT - 1 else PART
                    zt = sp.tile([PART, NCLS], F32, tag="es", name="zt")
                    nc.vector.memset(zt[:], 0.0)
                    nc.gpsimd.dma_start(out[t * PART:t * PART + vp, :],
                                        zt[0:vp, :])

            # ---------------- generic sparse hop
            def hop(tbl_in, nfree, on_block_done):
                tA = tbl_in[0:cfg.TA_H, :]
                tB = tbl_in[cfg.B_BASE:N, :]
                for bi, bt in enumerate(g.batches):
                    q = bi % NQ
                    gA = gB = None
                    if bt["ncA"]:
                        gA = gp.tile([PART, bt["ncA"], COUT], BF16, tag="g")
                        nc.gpsimd.dma_gather(
                            gA[:], tA,
                            idx_sb[:, bt["idx_colA"]:
                                   bt["idx_colA"] + bt["ncA"] * 8],
                            bt["ncA"] * PART, bt["ncA"] * PART, COUT,
                            single_packet=False, queue_num=q)
                    if bt["ncB"]:
                        gB = gp.tile([PART, bt["ncB"], COUT], BF16, tag="g")
                        nc.gpsimd.dma_gather(
                            gB[:], tB,
                            idx_sb[:, bt["idx_colB"]:
                                   bt["idx_colB"] + bt["ncB"] * 8],
                            bt["ncB"] * PART, bt["ncB"] * PART, COUT,
                            single_packet=False, queue_num=q)
                    cur_b = -1
                    psum = None
                    for c in bt["chunks"]:
                        if c["b"] != cur_b:
                            if cur_b >= 0:
                                on_block_done(cur_b, psum)
                            cur_b = c["b"]
                            if nfree == COUT:
                                psum = pb.tile([PART, COUT], F32, tag="big",
                                               name="psum_hop")
                            else:
                                psum = ph.tile([PART, 1], F32, tag="hd",
                                               name="psum_hd")
                        woff = (c["w"] % cfg.WPB) * W
                        gt = gA if c["tab"] == 0 else gB
                        nc.tensor.matmul(
                            psum[woff:woff + W, 0:nfree],
                            s_sb[:, c["g"], :],
                            gt[:, c["pos"], 0:nfree],
                            start=c["first"], stop=c["last"],
                            skip_group_check=True,
                            tile_position=(0, woff))
                    if cur_b >= 0:
                        on_block_done(cur_b, psum)

            if STAGE < 1:
                _early_out()
                _finalize(nc)
                return nc

            # hop 1: T1
            def done1(b, psum):
                vp = LT if b == NT - 1 else PART
                nc.scalar.copy(t1_sb[0:vp, b, :], psum[0:vp, :])
            hop(tbl[0], COUT, done1)
            store_shard(agin[1], t1_sb)
            nc.gpsimd.collective_compute("AllGather", ALU.bypass, RG,
                                         ins=[agin[1][:]], outs=[tbl[1][:]])

            if STAGE < 2:
                _early_out()
                _finalize(nc)
                return nc

            # hop 2: T2 (db recomputed in the epilogue)
            def done2(b, psum):
                vp = LT if b == NT - 1 else PART
                nc.scalar.copy(t2_sb[0:vp, b, :], psum[0:vp, :])
            hop(tbl[1], COUT, done2)
            store_shard(agin[2], t2_sb)
            nc.gpsimd.collective_compute("AllGather", ALU.bypass, RG,
                                         ins=[agin[2][:]], outs=[tbl[2][:]])

            if STAGE < 3:
                _early_out()
                _finalize(nc)
                return nc

            # hop 3: u = T3 + c2L*T2 + c1L*T1 + (c0L-1)*h ; dn2 = row sumsq
            def done3(b, psum):
                vp = LT if b == NT - 1 else PART
                e1 = sp.tile([PART, COUT], F32, tag="e1")
                nc.vector.scalar_tensor_tensor(
                    e1[0:vp, :], t2_sb[0:vp, b, :], C2L[0:vp, :],
                    psum[0:vp, :], ALU.mult, ALU.add)
                nc.vector.scalar_tensor_tensor(
                    e1[0:vp, :], t1_sb[0:vp, b, :], C1L[0:vp, :],
                    e1[0:vp, :], ALU.mult, ALU.add)
                nc.vector.scalar_tensor_tensor(
                    u_sb[0:vp, b, :], h_sb[0:vp, b, :], C0L1[0:vp, :],
                    e1[0:vp, :], ALU.mult, ALU.add)
                sq = sp.tile([PART, COUT], F32, tag="sq")
                nc.scalar.activation(sq[0:vp, :], u_sb[0:vp, b, :], AF.Square,
                                     accum_out=dn_sb[0:vp, b:b + 1])
            hop(tbl[2], COUT, done3)

            if STAGE < 4:
                _early_out()
                _finalize(nc)
                return nc

            # dn = sqrt(dn2); dnb table rows = dn broadcast over channels
            nc.scalar.sqrt(dn_sb[:], dn_sb[:])
            for t in range(NT):
                vp = LT if t == NT - 1 else PART
                dnb_t = sp.tile([PART, COUT], BF16, tag="fr")
                nc.vector.tensor_scalar_mul(dnb_t[0:vp, :], onesCb[0:vp, :],
                                            dn_sb[0:vp, t:t + 1])
                nc.sync.dma_start(agin[3][t * PART:t * PART + vp, :],
                                  dnb_t[0:vp, :])
            nc.gpsimd.collective_compute("AllGather", ALU.bypass, RG,
                                         ins=[agin[3][:]], outs=[tbl[3][:]])

            if STAGE < 5:
                _early_out()
                _finalize(nc)
                return nc

            # hop 4: hd = L @ dn
            def done4(b, psum):
                vp = LT if b == NT - 1 else PART
                nc.scalar.copy(hd_sb[0:vp, b:b + 1], psum[0:vp, 0:1])
            hop(tbl[3], 1, done4)

            if STAGE < 6:
                _early_out()
                _finalize(nc)
                return nc

            # ---------------- global min/max -> AllReduce(max) of [max, -min]
            mx1 = sp.tile([PART, 1], F32, tag="mm")
            mn1 = sp.tile([PART, 1], F32, tag="mm")
            if NT > 1:
                nc.vector.reduce_max(mx1[:], hd_sb[:, 0:NT - 1], axis=AXL.X)
                nc.vector.tensor_reduce(mn1[:], hd_sb[:, 0:NT - 1], axis=AXL.X, op=ALU.min)
                nc.vector.tensor_tensor(mx1[0:LT, :], mx1[0:LT, :],
                                        hd_sb[0:LT, NT - 1:NT], op=ALU.max)
                nc.vector.tensor_tensor(mn1[0:LT, :], mn1[0:LT, :],
                                        hd_sb[0:LT, NT - 1:NT], op=ALU.min)
            else:
                nc.vector.reduce_max(mx1[0:LT, :], hd_sb[0:LT, :], axis=AXL.X)
                nc.vector.tensor_reduce(mn1[0:LT, :], hd_sb[0:LT, :],
                                        axis=AXL.X, op=ALU.min)
            nc.vector.tensor_scalar_mul(mn1[:], mn1[:], -1.0)
            pmx = ps.tile([PART, PART], F32, tag="sm")
            transpose(pmx[0:1, 0:PART], mx1[:], id_f32, PART)
            pmn = ps.tile([PART, PART], F32, tag="sm")
            transpose(pmn[0:1, 0:PART], mn1[:], id_f32, PART)
            mm_sb = sp.tile([1, 2], F32, tag="mm2")
            nc.vector.reduce_max(mm_sb[0:1, 0:1], pmx[0:1, 0:PART], axis=AXL.X)
            nc.vector.reduce_max(mm_sb[0:1, 1:2], pmn[0:1, 0:PART], axis=AXL.X)
            nc.sync.dma_start(mm_in[:], mm_sb[:])
            nc.gpsimd.collective_compute("AllReduce", ALU.max, RG,
                                         ins=[mm_in[:]], outs=[mm_out[:]])
            mmg = sp.tile([1, 2], F32, tag="mm2")
            nc.sync.dma_start(mmg[:], mm_out[:])

            # s = 2a/(mx - mn); ns = (hd + (-mn)) * s  (= normal * 2a)
            sc = cp.tile([1, 2], F32)
            nc.vector.tensor_add(sc[:, 0:1], mmg[:, 0:1], mmg[:, 1:2])
            nc.vector.reciprocal(sc[:, 0:1], sc[:, 0:1])
            nc.vector.tensor_mul(sc[:, 0:1], sc[:, 0:1], av_sb[:])
            nc.vector.tensor_scalar_mul(sc[:, 0:1], sc[:, 0:1], 2.0)
            nc.vector.tensor_copy(sc[:, 1:2], mmg[:, 1:2])
            pbc = ps.tile([PART, PART], F32, tag="sm")
            nc.tensor.matmul(pbc[0:PART, 0:2], ones1f[:], sc[:],
                             start=True, stop=True, skip_group_check=True)
            bc = cp.tile([PART, 2], F32)
            nc.scalar.copy(bc[:], pbc[0:PART, 0:2])
            nc.vector.tensor_scalar(ns_sb[:], hd_sb[:], bc[:, 1:2],
                                    bc[:, 0:1], ALU.add, ALU.mult)

            if STAGE < 7:
                _early_out()
                _finalize(nc)
                return nc

            # ---------------- epilogue (phase-split to avoid act-table thrash)
            pys = rp.tile([PART, NT, NCLS], F32)
            nm_sb = rp.tile([PART, NT], F32)
            ss_sb = rp.tile([PART, NT], F32)
            lse_sb = rp.tile([PART, NT], F32)
            nc.vector.memset(ss_sb[:], 1.0)
            for t in range(NT):
                vp = LT if t == NT - 1 else PART
                e = sp.tile([PART, COUT], F32, tag="e1")
                nc.vector.scalar_tensor_tensor(
                    e[0:vp, :], t1_sb[0:vp, t, :], N2D[0:vp, :],
                    t2_sb[0:vp, t, :], ALU.mult, ALU.add)
                nc.vector.scalar_tensor_tensor(
                    e[0:vp, :], h_sb[0:vp, t, :], D2C[0:vp, :],
                    e[0:vp, :], ALU.mult, ALU.add)
                f1 = sp.tile([PART, COUT], F32, tag="f1")
                nc.vector.scalar_tensor_tensor(
                    f1[0:vp, :], e[0:vp, :], ns_sb[0:vp, t:t + 1],
                    u_sb[0:vp, t, :], ALU.mult, ALU.add)
                nc.vector.tensor_add(f1[0:vp, :], f1[0:vp, :],
                                     h_sb[0:vp, t, :])
                fr = sp.tile([PART, COUT], BF16, tag="fr")
                nc.scalar.activation(fr[0:vp, :], f1[0:vp, :], AF.Relu)
                ptr = pb.tile([PART, PART], BF16, tag="big")
                transpose(ptr[0:COUT, 0:vp], fr[0:vp, :], id_bf, vp)
                frT = sp.tile([PART, PART], BF16, tag="frT")
                nc.vector.tensor_copy(frT[0:COUT, 0:vp], ptr[0:COUT, 0:vp])
                py = ps.tile([PART, NCLS], F32, tag="sm")
                nc.tensor.matmul(py[0:vp, :], frT[:, 0:vp], Wo_sb[:],
                                 start=True, stop=False)
                nc.tensor.matmul(py[0:vp, :], ones1b[:, 0:vp], bo_sb[:],
                                 start=False, stop=True)
                nc.vector.tensor_copy(pys[0:vp, t, :], py[0:vp, :])
                nc.vector.reduce_max(nm_sb[0:vp, t:t + 1], py[0:vp, :],
                                     axis=AXL.X, negate=True)
            for t in range(NT):
                vp = LT if t == NT - 1 else PART
                es = sp.tile([PART, NCLS], F32, tag="es")
                nc.scalar.activation(es[0:vp, :], pys[0:vp, t, :], AF.Exp,
                                     bias=nm_sb[0:vp, t:t + 1],
                                     accum_out=ss_sb[0:vp, t:t + 1])
            nc.scalar.activation(lse_sb[:], ss_sb[:], AF.Ln)
            for t in range(NT):
                vp = LT if t == NT - 1 else PART
                ot = sp.tile([PART, NCLS], F32, tag="es")
                nc.vector.tensor_scalar(ot[0:vp, :], pys[0:vp, t, :],
                                        nm_sb[0:vp, t:t + 1],
                                        lse_sb[0:vp, t:t + 1],
                                        ALU.add, ALU.subtract)
                r0 = t * PART
                nc.sync.dma_start(out[r0:r0 + vp, :], ot[0:vp, :])

    _finalize(nc)
    return nc


# ---------------------------------------------------------------------------
# entry point
# ---------------------------------------------------------------------------

def _in_maps(cfg, g, x, W_in, b_in, W_out, b_out, delta, a):
    ident = np.eye(PART, dtype=np.float32).astype(ml_dtypes.bfloat16)
    maps = []
    for m in range(PCORES):
        maps.append({
            "xs": np.ascontiguousarray(
                x[m * cfg.SHARD:(m + 1) * cfg.SHARD].T).astype(
                    ml_dtypes.bfloat16),
            "Wi": np.ascontiguousarray(W_in).astype(np.float32),
            "bi": b_in.reshape(1, -1).astype(np.float32),
            "Wo": np.ascontiguousarray(W_out).astype(np.float32),
            "bo": b_out.reshape(1, -1).astype(np.float32),
            "dl": delta.reshape(1, 1).astype(np.float32),
            "av": a.reshape(1, 1).astype(np.float32),
            "idt": ident,
            "sblob": g.sblobs[m],
            "iblob": g.iblobs[m],
        })
    return maps


def prepare(x, vals, W_in, b_in, delta, a, W_out, b_out, rows, cols,
            debug=False, **cfg_kw):
    x = np.asarray(x)
    cfg = Cfg(N=x.shape[0], E=len(np.asarray(vals)), CIN=x.shape[1],
              COUT=np.asarray(W_in).shape[1], NCLS=np.asarray(W_out).shape[1],
              **cfg_kw)
    g = build_grid(cfg, np.asarray(rows), np.asarray(cols),
                   np.asarray(vals, np.float32))
    nc = build_nc(cfg, g, debug=debug)
    maps = _in_maps(cfg, g, x, np.asarray(W_in), np.asarray(b_in),
                    np.asarray(W_out), np.asarray(b_out),
                    np.asarray(delta), np.asarray(a))
    return cfg, g, nc, maps


def kernel(x, vals, W_in, b_in, delta, a, W_out, b_out, rows, cols):
    from concourse.bass_utils import run_bass_kernel_spmd

    cfg, g, nc, maps = prepare(x, vals, W_in, b_in, delta, a, W_out, b_out,
                               rows, cols)
    res = run_bass_kernel_spmd(nc, maps, core_ids=list(range(PCORES)))
    return np.concatenate([res.results[m]["out"] for m in range(PCORES)], 0)



# revision 23
# speedup vs baseline: 2.1782x; 1.0001x over previous
"""AnomalyNet (3-hop Chebyshev-style GNN) on 8 Trainium2 NeuronCores.

Strategy:
  - Shard nodes (rows) across 8 cores: core m owns rows [m*SHARD, (m+1)*SHARD).
  - Dense parts (x @ W_in, epilogue @ W_out) on TensorE, bf16.
  - Each spmm hop: the full feature table [N, 128] bf16 lives in HBM
    (AllGather of per-core shard outputs).  Each core gathers its edges'
    source rows with dma_gather (one 256B descriptor per edge), and reduces
    into rows via PE "segment matmuls": for each chunk of 128 edge slots, a
    host-built scatter matrix S [128 slots, 32 window-rows] (vals baked in,
    bf16) is the stationary operand; out[window, :] += S^T @ gathered.
  - int16 gather indices cap the table height at 32768, so gathers read one
    of two overlapping views of the table (rows [0, 32768) / [N-32768, N));
    edges in the overlap are assigned to balance chunk counts.
  - The chunk grid (windows x chunk counts) is shared by all 8 cores (SPMD:
    one instruction stream); per-core edge data (indices, S values) comes in
    via per-core input blobs.  Pad slots point at a real row with val 0.
  - 4th sparse pass (high_delta = L @ dn) reuses the same chunks/indices with
    a table of dn broadcast across 128 channels.
  - Global min/max of high_delta via a 2-element AllReduce(max) of
    [max, -min].
"""

import math
import os

import numpy as np
import ml_dtypes

import concourse.bacc as bacc
import concourse.bass as bass
import concourse.mybir as mybir
import concourse.tile as tile

F32 = mybir.dt.float32
BF16 = mybir.dt.bfloat16
I16 = mybir.dt.int16
AF = mybir.ActivationFunctionType
ALU = mybir.AluOpType
AXL = mybir.AxisListType

PCORES = 8
PART = 128  # partitions


class Cfg:
    def __init__(self, N=50000, E=800000, CIN=256, COUT=128, NCLS=7,
                 W=32, NBB=2, IDX_CAP=32768):
        assert N % PCORES == 0
        self.N, self.E, self.CIN, self.COUT, self.NCLS = N, E, CIN, COUT, NCLS
        self.SHARD = N // PCORES
        self.W = W                      # window rows per chunk
        self.WPB = PART // W            # windows per 128-row block
        self.NT = math.ceil(self.SHARD / PART)   # 128-row blocks per core
        self.LT = self.SHARD - (self.NT - 1) * PART  # rows in last block
        self.NW = math.ceil(self.SHARD / W)      # windows per core
        self.NBB = NBB                  # blocks per gather batch
        self.TA_H = min(N, IDX_CAP)     # table A = rows [0, TA_H)
        self.B_BASE = max(0, N - IDX_CAP)  # table B = rows [B_BASE, N)
        self.CINT = math.ceil(CIN / PART)  # 128-col tiles of CIN
        self.NTP = math.ceil(self.SHARD / 16) * 16  # shard rows padded (xbar)


# ---------------------------------------------------------------------------
# host-side preprocessing: build the (core-uniform) chunk grid and per-core
# index / S blobs
# ---------------------------------------------------------------------------

class Grid:
    """Uniform structure shared by all cores + per-core data blobs."""
    pass


def build_grid(cfg: Cfg, rows: np.ndarray, cols: np.ndarray, vals: np.ndarray):
    N, W = cfg.N, cfg.W
    SHARD, NW = cfg.SHARD, cfg.NW

    core_of = rows // SHARD
    rloc = rows - core_of * SHARD
    wid = rloc // W

    # category: 0 = A-only (c < B_BASE), 1 = flex, 2 = B-only (c >= TA_H)
    cat = np.ones(cfg.E, np.int8)
    cat[cols < cfg.B_BASE] = 0
    cat[cols >= cfg.TA_H] = 2

    order = np.lexsort((cat, wid, core_of))
    counts = np.zeros((PCORES, NW, 3), np.int64)
    np.add.at(counts, (core_of, wid, cat), 1)
    starts = np.zeros((PCORES, NW, 3), np.int64)
    starts.reshape(-1)[1:] = np.cumsum(counts.reshape(-1))[:-1]

    nA0 = counts[:, :, 0]
    nF = counts[:, :, 1]
    nB0 = counts[:, :, 2]
    ntot = nA0 + nF + nB0

    # uniform chunk counts per window
    kA = np.ceil(nA0.max(0) / 128).astype(int)
    kB = np.ceil(nB0.max(0) / 128).astype(int)
    need = np.ceil(ntot.max(0) / 128).astype(int)
    for w in range(NW):
        while kA[w] + kB[w] < max(need[w], 1):
            slackA = kA[w] * 128 - nA0[:, w].max()
            slackB = kB[w] * 128 - nB0[:, w].max()
            if kB[w] == 0 or slackA <= slackB:
                kA[w] += 1
            else:
                kB[w] += 1
    assert (kA[None, :] * 128 >= nA0).all()
    assert (kB[None, :] * 128 >= nB0).all()
    assert ((kA + kB)[None, :] * 128 >= ntot).all()

    g = Grid()
    g.kA, g.kB = kA, kB
    g.nchunks = int((kA + kB).sum())

    # chunk gidx base per window (A-chunks then B-chunks, windows in order)
    wbaseA = np.zeros(NW, np.int64)
    wbaseB = np.zeros(NW, np.int64)
    base = 0
    for w in range(NW):
        wbaseA[w] = base
        wbaseB[w] = base + kA[w]
        base += kA[w] + kB[w]

    # batches of NBB blocks; within a batch: chunks ordered by block, window,
    # A-chunks then B-chunks.  Gather call A covers the batch's A-chunks in
    # that order; call B likewise.
    nbatch = math.ceil(cfg.NT / cfg.NBB)
    batches = []
    idx_cols_total = 0
    for bi in range(nbatch):
        blocks = range(bi * cfg.NBB, min((bi + 1) * cfg.NBB, cfg.NT))
        ch = []
        for b in blocks:
            for w in range(b * cfg.WPB, min((b + 1) * cfg.WPB, NW)):
                nw_ch = int(kA[w] + kB[w])
                for i in range(nw_ch):
                    gi = int((wbaseA[w] if i < kA[w] else wbaseB[w] - kA[w])
                             + i)
                    ch.append(dict(g=gi, w=w, tab=0 if i < kA[w] else 1, b=b,
                                   first=(i == 0), last=(i == nw_ch - 1)))
        ncA = sum(1 for c in ch if c["tab"] == 0)
        ncB = len(ch) - ncA
        pA = pB = 0
        for c in ch:
            if c["tab"] == 0:
                c["pos"] = pA
                pA += 1
            else:
                c["pos"] = pB
                pB += 1
        batches.append(dict(blocks=list(blocks), ncA=ncA, ncB=ncB, chunks=ch,
                            idx_colA=idx_cols_total,
                            idx_colB=idx_cols_total + ncA * 8))
        idx_cols_total += (ncA + ncB) * 8
    g.batches = batches
    g.idx_cols_total = idx_cols_total

    # ---- per-core blobs
    g.sblobs = []
    g.iblobs = []
    for m in range(PCORES):
        S = np.zeros((g.nchunks, PART, W), np.float32)
        idx_all = np.zeros((g.nchunks, PART), np.int16)

        for w in range(NW):
            e0a, n0a = starts[m, w, 0], counts[m, w, 0]
            e0f, n0f = starts[m, w, 1], counts[m, w, 1]
            e0b, n0b = starts[m, w, 2], counts[m, w, 2]
            capA = kA[w] * 128
            fA = min(n0f, capA - n0a)
            assert n0b + (n0f - fA) <= kB[w] * 128

            selA = np.concatenate([order[e0a:e0a + n0a], order[e0f:e0f + fA]])
            selB = np.concatenate([order[e0f + fA:e0f + n0f],
                                   order[e0b:e0b + n0b]])
            for sel, k, gbase, off in (
                (selA, int(kA[w]), int(wbaseA[w]), 0),
                (selB, int(kB[w]), int(wbaseB[w]), cfg.B_BASE),
            ):
                ns = len(sel)
                if k == 0:
                    assert ns == 0
                    continue
                ids = (cols[sel] - off).astype(np.int16)
                vs = vals[sel]
                rr = (rloc[sel] - w * W).astype(np.int64)
                cap = k * 128
                pad_idx = ids[-1] if ns else np.int16(0)
                full_ids = np.full(cap, pad_idx, np.int16)
                full_ids[:ns] = ids
                for j in range(k):
                    gi = gbase + j
                    seg = np.arange(j * 128, min((j + 1) * 128, ns))
                    if len(seg):
                        S[gi, seg - j * 128, rr[seg]] = vs[seg]
                    idx_all[gi] = full_ids[j * 128:(j + 1) * 128]

        iblob = np.zeros((16, idx_cols_total), np.int16)
        for bt in batches:
            for c in bt["chunks"]:
                col0 = (bt["idx_colA"] if c["tab"] == 0 else bt["idx_colB"]) \
                    + c["pos"] * 8
                iblob[:, col0:col0 + 8] = idx_all[c["g"]].reshape(8, 16).T
        g.sblobs.append(
            np.ascontiguousarray(S.transpose(1, 0, 2)).astype(
                ml_dtypes.bfloat16))
        g.iblobs.append(iblob)
    return g


# ---------------------------------------------------------------------------
# kernel builder (SPMD graph, shared by all cores)
# ---------------------------------------------------------------------------

def _finalize(nc):
    nc.compile()


# Bacc.compile()'s late passes (library/act-table loads, hostgen rebases) run
# after its last generate_event_semaphores() and can leave DMA instructions
# with 2 sync waits; walrus's DIRECT2D lowering has a single wait slot and
# dies with "Too many sync wait commands".  Splice one more splitter pass in
# right before ISA codegen (the last point where inserted EventSemaphores
# still go through codegen).
import concourse.bacc as _bacc_mod

if not getattr(_bacc_mod.Bacc, "_evsem_patch", False):
    _orig_codegen = _bacc_mod.Bacc.codegen_inst_isa_subclasses

    def _codegen_with_split(self):
        self.generate_event_semaphores()
        return _orig_codegen(self)

    _bacc_mod.Bacc.codegen_inst_isa_subclasses = _codegen_with_split
    _bacc_mod.Bacc._evsem_patch = True


def build_nc(cfg: Cfg, g, debug=False):
    STAGE = int(os.environ.get("KSTAGE", "99"))
    NQ = int(os.environ.get("KQ", "4"))  # SWDGE queues for gather desc-gen
    nc = bacc.Bacc("TRN2", target_bir_lowering=False, debug=debug,
                   num_devices=PCORES, num_swdge_queues=NQ,
                   dynamic_dma_scratch_size=16384)
    N, COUT, NCLS, W = cfg.N, cfg.COUT, cfg.NCLS, cfg.W
    SHARD, NT, LT = cfg.SHARD, cfg.NT, cfg.LT
    RG = [list(range(PCORES))]

    # ---------------- dram parameters
    xs = nc.declare_dram_parameter("xs", [cfg.CIN, SHARD], BF16, isOutput=False)
    Wi = nc.declare_dram_parameter("Wi", [cfg.CIN, COUT], F32, isOutput=False)
    bi = nc.declare_dram_parameter("bi", [1, COUT], F32, isOutput=False)
    Wo = nc.declare_dram_parameter("Wo", [COUT, NCLS], F32, isOutput=False)
    bo = nc.declare_dram_parameter("bo", [1, NCLS], F32, isOutput=False)
    dl = nc.declare_dram_parameter("dl", [1, 1], F32, isOutput=False)
    av = nc.declare_dram_parameter("av", [1, 1], F32, isOutput=False)
    idt = nc.declare_dram_parameter("idt", [PART, PART], BF16, isOutput=False)
    sbl = nc.declare_dram_parameter("sblob", [PART, g.nchunks, W], BF16,
                                    isOutput=False)
    ibl = nc.declare_dram_parameter("iblob", [16, g.idx_cols_total], I16,
                                    isOutput=False)
    out = nc.declare_dram_parameter("out", [SHARD, NCLS], F32, isOutput=True)

    # ---------------- internal dram
    agin = [nc.dram_tensor(f"agin{k}", [SHARD, COUT], BF16) for k in range(3)]
    tbl = [nc.dram_tensor(f"tbl{k}", [N, COUT], BF16, addr_space="Shared")
           for k in range(4)]
    dn_in = nc.dram_tensor("dn_in", [SHARD, 1], BF16)
    dn_full = nc.dram_tensor("dn_full", [N, 1], BF16, addr_space="Shared")
    mm_in = nc.dram_tensor("mm_in", [1, 2], F32)
    mm_out = nc.dram_tensor("mm_out", [1, 2], F32, addr_space="Shared")

    def store_shard(dram, sb3):
        """sb3 [128, NT, COUT] -> dram [SHARD, COUT], row = t*128 + p."""
        if NT > 1:
            nc.sync.dma_start(
                dram[0:(NT - 1) * PART, :].rearrange("(t p) c -> p t c",
                                                     p=PART),
                sb3[:, 0:NT - 1, :])
        nc.sync.dma_start(dram[(NT - 1) * PART:SHARD, :], sb3[0:LT, NT - 1, :])

    def transpose(out_ap, in_ap, ident, k):
        nc.tensor.matmul(out_ap, in_ap, ident[0:k, 0:k], is_transpose=True,
                         start=True, stop=True, skip_group_check=True)

    with tile.TileContext(nc) as tc:
        with (
            tc.tile_pool(name="const", bufs=1) as cp,
            tc.tile_pool(name="resid", bufs=1) as rp,
            tc.tile_pool(name="gat", bufs=2 * NQ) as gp,
            tc.tile_pool(name="stage", bufs=3) as sp,
            tc.tile_pool(name="pbig", bufs=4, space="PSUM") as pb,
            tc.tile_pool(name="psmall", bufs=2, space="PSUM") as ps,
            tc.tile_pool(name="phd", bufs=2, space="PSUM") as ph,
        ):
            # ---------------- constants
            Wi_sb = cp.tile([PART, cfg.CINT, COUT], BF16)
            for h in range(cfg.CINT):
                hi = min(cfg.CIN - h * PART, PART)
                nc.gpsimd.dma_start(Wi_sb[0:hi, h, :],
                                    Wi[h * PART:h * PART + hi, :])
            bi_sb = cp.tile([1, COUT], BF16)
            nc.gpsimd.dma_start(bi_sb[:], bi[:])
            Wo_sb = cp.tile([PART, NCLS], BF16)
            nc.gpsimd.dma_start(Wo_sb[0:COUT, :], Wo[:])
            bo_sb = cp.tile([1, NCLS], BF16)
            nc.gpsimd.dma_start(bo_sb[:], bo[:])
            id_bf = cp.tile([PART, PART], BF16)
            nc.sync.dma_start(id_bf[:], idt[:])
            id_f32 = cp.tile([PART, PART], F32)
            nc.vector.tensor_copy(id_f32[:], id_bf[:])
            ones1b = cp.tile([1, PART], BF16)
            nc.vector.memset(ones1b[:], 1.0)
            ones1f = cp.tile([1, PART], F32)
            nc.vector.memset(ones1f[:], 1.0)
            dl_sb = cp.tile([1, 1], F32)
            nc.sync.dma_start(dl_sb[:], dl[:])
            av_sb = cp.tile([1, 1], F32)
            nc.sync.dma_start(av_sb[:], av[:])

            idx_sb = cp.tile([PART, g.idx_cols_total], I16)
            for k in range(8):
                nc.sync.dma_start(idx_sb[16 * k:16 * (k + 1), :], ibl[:])
            s_sb = cp.tile([PART, g.nchunks, W], BF16)
            nc.sync.dma_start(s_sb[:], sbl[:])

            # resident feature tiles [128, NT, COUT]
            h_sb = rp.tile([PART, NT, COUT], BF16)
            t1_sb = rp.tile([PART, NT, COUT], BF16)
            t2_sb = rp.tile([PART, NT, COUT], BF16)
            u_sb = rp.tile([PART, NT, COUT], BF16)
            dn_sb = rp.tile([PART, NT], F32)
            nc.vector.memset(dn_sb[:], 0.0)
            hd_sb = rp.tile([PART, NT], F32)
            ns_sb = rp.tile([PART, NT], F32)

            # ---------------- coefficients (f32 [1,1] lane ops)
            cofp = cp.tile([1, 8], F32)  # c2L c1L c0L1 n2d d2
            nc.vector.memset(cofp[:], 0.0)
            t_ = cp.tile([1, 6], F32)
            d_, a_ = dl_sb[:], av_sb[:]
            d2_ = t_[:, 0:1]
            nc.vector.tensor_mul(d2_, d_, d_)
            d3_ = t_[:, 1:2]
            nc.vector.tensor_mul(d3_, d2_, d_)
            da_ = t_[:, 2:3]
            nc.vector.tensor_mul(da_, d_, a_)
            d2a_ = t_[:, 3:4]
            nc.vector.tensor_mul(d2a_, d2_, a_)
            nc.vector.scalar_tensor_tensor(cofp[:, 0:1], d_, -3.0, a_,
                                           ALU.mult, ALU.subtract)  # -3d - a
            t30 = t_[:, 4:5]
            nc.vector.tensor_scalar_mul(t30, d2_, 3.0)
            nc.vector.scalar_tensor_tensor(cofp[:, 1:2], da_, 2.0, t30,
                                           ALU.mult, ALU.add)  # 3d2 + 2da
            t31 = t_[:, 5:6]
            nc.vector.tensor_add(t31, d3_, d2a_)
            nc.vector.tensor_scalar(cofp[:, 2:3], t31, -1.0, -1.0,
                                    ALU.mult, ALU.add)  # -(d3 + d2a) - 1
            nc.vector.tensor_scalar_mul(cofp[:, 3:4], d_, -2.0)  # -2d
            nc.vector.tensor_copy(cofp[:, 4:5], d2_)  # d2
            pco = ps.tile([PART, 8], F32, tag="sm")
            nc.tensor.matmul(pco[:], ones1f[:], cofp[:], start=True, stop=True)
            cf_sb = cp.tile([PART, 8], F32)
            nc.scalar.copy(cf_sb[:], pco[:])
            C2L, C1L, C0L1 = cf_sb[:, 0:1], cf_sb[:, 1:2], cf_sb[:, 2:3]
            N2D, D2C = cf_sb[:, 3:4], cf_sb[:, 4:5]

            # ---------------- prologue: h = tanh(x @ Wi + bi)
            with tc.tile_pool(name="xt", bufs=3) as xp:
                for t in range(NT):
                    vp = LT if t == NT - 1 else PART
                    r0 = t * PART
                    xTt = xp.tile([PART, cfg.CINT, PART], BF16,
                                  name="xTt", tag="xTt")
                    for h in range(cfg.CINT):
                        hi = min(cfg.CIN - h * PART, PART)
                        nc.sync.dma_start(
                            xTt[0:hi, h, 0:vp],
                            xs[h * PART:h * PART + hi, r0:r0 + vp])
                    pt = pb.tile([PART, COUT], F32, tag="big")
                    for h in range(cfg.CINT):
                        nc.tensor.matmul(pt[0:vp, :], xTt[:, h, 0:vp],
                                         Wi_sb[:, h, :], start=(h == 0),
                                         stop=False)
                    nc.tensor.matmul(pt[0:vp, :], ones1b[:, 0:vp], bi_sb[:],
                                     start=False, stop=True)
                    nc.scalar.activation(h_sb[0:vp, t, :], pt[0:vp, :],
                                         AF.Tanh)
            store_shard(agin[0], h_sb)
            nc.gpsimd.collective_compute("AllGather", ALU.bypass, RG,
                                         ins=[agin[0][:]], outs=[tbl[0][:]])

            def _early_out():
                for t in range(NT):
                    vp = LT if t == NT - 1 else PART
                    zt = sp.tile([PART, NCLS], F32, tag="es", name="zt")
                    nc.vector.memset(zt[:], 0.0)
                    nc.gpsimd.dma_start(out[t * PART:t * PART + vp, :],
                                        zt[0:vp, :])

            # ---------------- generic sparse hop
            def hop(tbl_in, nfree, on_block_done):
                tA = tbl_in[0:cfg.TA_H, :]
                tB = tbl_in[cfg.B_BASE:N, :]
                for bi, bt in enumerate(g.batches):
                    q = bi % NQ
                    gA = gB = None
                    if bt["ncA"]:
                        gA = gp.tile([PART, bt["ncA"], COUT], BF16, tag="g")
                        nc.gpsimd.dma_gather(
                            gA[:], tA,
                            idx_sb[:, bt["idx_colA"]:
                                   bt["idx_colA"] + bt["ncA"] * 8],
                            bt["ncA"] * PART, bt["ncA"] * PART, COUT,
                            single_packet=False, queue_num=q)
                    if bt["ncB"]:
                        gB = gp.tile([PART, bt["ncB"], COUT], BF16, tag="g")
                        nc.gpsimd.dma_gather(
                            gB[:], tB,
                            idx_sb[:, bt["idx_colB"]:
                                   bt["idx_colB"] + bt["ncB"] * 8],
                            bt["ncB"] * PART, bt["ncB"] * PART, COUT,
                            single_packet=False, queue_num=q)
                    cur_b = -1
                    psum = None
                    for c in bt["chunks"]:
                        if c["b"] != cur_b:
                            if cur_b >= 0:
                                on_block_done(cur_b, psum)
                            cur_b = c["b"]
                            if nfree == COUT:
                                psum = pb.tile([PART, COUT], F32, tag="big",
                                               name="psum_hop")
                            else:
                                psum = ph.tile([PART, 1], F32, tag="hd",
                                               name="psum_hd")
                        woff = (c["w"] % cfg.WPB) * W
                        gt = gA if c["tab"] == 0 else gB
                        nc.tensor.matmul(
                            psum[woff:woff + W, 0:nfree],
                            s_sb[:, c["g"], :],
                            gt[:, c["pos"], 0:nfree],
                            start=c["first"], stop=c["last"],
                            skip_group_check=True,
                            tile_position=(0, woff))
                    if cur_b >= 0:
                        on_block_done(cur_b, psum)

            if STAGE < 1:
                _early_out()
                _finalize(nc)
                return nc

            # hop 1: T1
            def done1(b, psum):
                vp = LT if b == NT - 1 else PART
                nc.scalar.copy(t1_sb[0:vp, b, :], psum[0:vp, :])
            hop(tbl[0], COUT, done1)
            store_shard(agin[1], t1_sb)
            nc.gpsimd.collective_compute("AllGather", ALU.bypass, RG,
                                         ins=[agin[1][:]], outs=[tbl[1][:]])

            if STAGE < 2:
                _early_out()
                _finalize(nc)
                return nc

            # hop 2: T2 (db recomputed in the epilogue)
            def done2(b, psum):
                vp = LT if b == NT - 1 else PART
                nc.scalar.copy(t2_sb[0:vp, b, :], psum[0:vp, :])
            hop(tbl[1], COUT, done2)
            store_shard(agin[2], t2_sb)
            nc.gpsimd.collective_compute("AllGather", ALU.bypass, RG,
                                         ins=[agin[2][:]], outs=[tbl[2][:]])

            if STAGE < 3:
                _early_out()
                _finalize(nc)
                return nc

            # hop 3: u = T3 + c2L*T2 + c1L*T1 + (c0L-1)*h ; dn2 = row sumsq
            def done3(b, psum):
                vp = LT if b == NT - 1 else PART
                e1 = sp.tile([PART, COUT], F32, tag="e1")
                nc.vector.scalar_tensor_tensor(
                    e1[0:vp, :], t2_sb[0:vp, b, :], C2L[0:vp, :],
                    psum[0:vp, :], ALU.mult, ALU.add)
                nc.vector.scalar_tensor_tensor(
                    e1[0:vp, :], t1_sb[0:vp, b, :], C1L[0:vp, :],
                    e1[0:vp, :], ALU.mult, ALU.add)
                nc.vector.scalar_tensor_tensor(
                    u_sb[0:vp, b, :], h_sb[0:vp, b, :], C0L1[0:vp, :],
                    e1[0:vp, :], ALU.mult, ALU.add)
                sq = sp.tile([PART, COUT], F32, tag="sq")
                nc.scalar.activation(sq[0:vp, :], u_sb[0:vp, b, :], AF.Square,
                                     accum_out=dn_sb[0:vp, b:b + 1])
                # t2 is no longer needed as-is: overwrite with
                # db = T2 - 2d*T1 + d2*h for the epilogue
                nc.vector.scalar_tensor_tensor(
                    t2_sb[0:vp, b, :], t1_sb[0:vp, b, :], N2D[0:vp, :],
                    t2_sb[0:vp, b, :], ALU.mult, ALU.add)
                nc.vector.scalar_tensor_tensor(
                    t2_sb[0:vp, b, :], h_sb[0:vp, b, :], D2C[0:vp, :],
                    t2_sb[0:vp, b, :], ALU.mult, ALU.add)
            hop(tbl[2], COUT, done3)

            if STAGE < 4:
                _early_out()
                _finalize(nc)
                return nc

            # dn = sqrt(dn2); AllGather the [N,1] dn vector (tiny), then
            # write it into column 0 of tbl[3] (the hop-4 gather reads only
            # column 0 of each 256B row; the rest stays garbage).
            nc.scalar.sqrt(dn_sb[:], dn_sb[:])
            dnb = sp.tile([PART, NT, 1], BF16, tag="dnb")
            nc.vector.tensor_copy(dnb[:, :, 0], dn_sb[:])
            if NT > 1:
                nc.sync.dma_start(
                    dn_in[0:(NT - 1) * PART, :].rearrange("(t p) c -> p t c",
                                                          p=PART),
                    dnb[:, 0:NT - 1, :])
            nc.sync.dma_start(dn_in[(NT - 1) * PART:SHARD, :],
                              dnb[0:LT, NT - 1, :])
            nc.gpsimd.collective_compute("AllGather", ALU.bypass, RG,
                                         ins=[dn_in[:]], outs=[dn_full[:]])
            NB_F = N // PART          # full 128-row groups of the dn vector
            NTL = N - NB_F * PART
            dnf = sp.tile([PART, NB_F + 1, 1], BF16, tag="dnf")
            nc.sync.dma_start(
                dnf[:, 0:NB_F, :],
                dn_full[0:NB_F * PART, :].rearrange("(b p) c -> p b c",
                                                    p=PART))
            if NTL:
                nc.sync.dma_start(dnf[0:NTL, NB_F, :],
                                  dn_full[NB_F * PART:N, :])
            nc.sync.dma_start(
                tbl[3][0:NB_F * PART, 0:1].rearrange("(b p) c -> p b c",
                                                     p=PART),
                dnf[:, 0:NB_F, :])
            if NTL:
                nc.sync.dma_start(tbl[3][NB_F * PART:N, 0:1],
                                  dnf[0:NTL, NB_F, :])

            if STAGE < 5:
                _early_out()
                _finalize(nc)
                return nc

            # hop 4: hd = L @ dn (vector is idle here: fold u <- u + h
            # so the epilogue's final mix is a single op per tile)
            def done4(b, psum):
                vp = LT if b == NT - 1 else PART
                nc.scalar.copy(hd_sb[0:vp, b:b + 1], psum[0:vp, 0:1])
                nc.vector.tensor_add(u_sb[0:vp, b, :], u_sb[0:vp, b, :],
                                     h_sb[0:vp, b, :])
            hop(tbl[3], 1, done4)

            if STAGE < 6:
                _early_out()
                _finalize(nc)
                return nc

            # ---------------- global min/max -> AllReduce(max) of [max, -min]
            mx1 = sp.tile([PART, 1], F32, tag="mm")
            mn1 = sp.tile([PART, 1], F32, tag="mm")
            if NT > 1:
                nc.vector.reduce_max(mx1[:], hd_sb[:, 0:NT - 1], axis=AXL.X)
                nc.vector.tensor_reduce(mn1[:], hd_sb[:, 0:NT - 1], axis=AXL.X, op=ALU.min)
                nc.vector.tensor_tensor(mx1[0:LT, :], mx1[0:LT, :],
                                        hd_sb[0:LT, NT - 1:NT], op=ALU.max)
                nc.vector.tensor_tensor(mn1[0:LT, :], mn1[0:LT, :],
                                        hd_sb[0:LT, NT - 1:NT], op=ALU.min)
            else:
                nc.vector.reduce_max(mx1[0:LT, :], hd_sb[0:LT, :], axis=AXL.X)
                nc.vector.tensor_reduce(mn1[0:LT, :], hd_sb[0:LT, :],
                                        axis=AXL.X, op=ALU.min)
            nc.vector.tensor_scalar_mul(mn1[:], mn1[:], -1.0)
            pmx = ps.tile([PART, PART], F32, tag="sm")
            transpose(pmx[0:1, 0:PART], mx1[:], id_f32, PART)
            pmn = ps.tile([PART, PART], F32, tag="sm")
            transpose(pmn[0:1, 0:PART], mn1[:], id_f32, PART)
            mm_sb = sp.tile([1, 2], F32, tag="mm2")
            nc.vector.reduce_max(mm_sb[0:1, 0:1], pmx[0:1, 0:PART], axis=AXL.X)
            nc.vector.reduce_max(mm_sb[0:1, 1:2], pmn[0:1, 0:PART], axis=AXL.X)
            nc.sync.dma_start(mm_in[:], mm_sb[:])
            nc.gpsimd.collective_compute("AllReduce", ALU.max, RG,
                                         ins=[mm_in[:]], outs=[mm_out[:]])
            mmg = sp.tile([1, 2], F32, tag="mm2")
            nc.sync.dma_start(mmg[:], mm_out[:])

            # s = 2a/(mx - mn); ns = (hd + (-mn)) * s  (= normal * 2a)
            sc = cp.tile([1, 2], F32)
            nc.vector.tensor_add(sc[:, 0:1], mmg[:, 0:1], mmg[:, 1:2])
            nc.vector.reciprocal(sc[:, 0:1], sc[:, 0:1])
            nc.vector.tensor_mul(sc[:, 0:1], sc[:, 0:1], av_sb[:])
            nc.vector.tensor_scalar_mul(sc[:, 0:1], sc[:, 0:1], 2.0)
            nc.vector.tensor_copy(sc[:, 1:2], mmg[:, 1:2])
            pbc = ps.tile([PART, PART], F32, tag="sm")
            nc.tensor.matmul(pbc[0:PART, 0:2], ones1f[:], sc[:],
                             start=True, stop=True, skip_group_check=True)
            bc = cp.tile([PART, 2], F32)
            nc.scalar.copy(bc[:], pbc[0:PART, 0:2])
            nc.vector.tensor_scalar(ns_sb[:], hd_sb[:], bc[:, 1:2],
                                    bc[:, 0:1], ALU.add, ALU.mult)

            if STAGE < 7:
                _early_out()
                _finalize(nc)
                return nc

            # ---------------- epilogue (phase-split to avoid act-table thrash)
            pys = rp.tile([PART, NT, NCLS], F32)
            nm_sb = rp.tile([PART, NT], F32)
            ss_sb = rp.tile([PART, NT], F32)
            lse_sb = rp.tile([PART, NT], F32)
            nc.vector.memset(ss_sb[:], 1.0)
            for t in range(NT):
                vp = LT if t == NT - 1 else PART
                # t2_sb holds db, u_sb holds u + h (precomputed in hops 3/4)
                f1 = sp.tile([PART, COUT], F32, tag="f1")
                nc.vector.scalar_tensor_tensor(
                    f1[0:vp, :], t2_sb[0:vp, t, :], ns_sb[0:vp, t:t + 1],
                    u_sb[0:vp, t, :], ALU.mult, ALU.add)
                fr = sp.tile([PART, COUT], BF16, tag="fr")
                nc.scalar.activation(fr[0:vp, :], f1[0:vp, :], AF.Relu)
                ptr = pb.tile([PART, PART], BF16, tag="big")
                transpose(ptr[0:COUT, 0:vp], fr[0:vp, :], id_bf, vp)
                frT = sp.tile([PART, PART], BF16, tag="frT")
                nc.vector.tensor_copy(frT[0:COUT, 0:vp], ptr[0:COUT, 0:vp])
                py = ps.tile([PART, NCLS], F32, tag="sm")
                nc.tensor.matmul(py[0:vp, :], frT[:, 0:vp], Wo_sb[:],
                                 start=True, stop=False)
                nc.tensor.matmul(py[0:vp, :], ones1b[:, 0:vp], bo_sb[:],
                                 start=False, stop=True)
                nc.vector.tensor_copy(pys[0:vp, t, :], py[0:vp, :])
                nc.vector.reduce_max(nm_sb[0:vp, t:t + 1], py[0:vp, :],
                                     axis=AXL.X, negate=True)
            for t in range(NT):
                vp = LT if t == NT - 1 else PART
                es = sp.tile([PART, NCLS], F32, tag="es")
                nc.scalar.activation(es[0:vp, :], pys[0:vp, t, :], AF.Exp,
                                     bias=nm_sb[0:vp, t:t + 1],
                                     accum_out=ss_sb[0:vp, t:t + 1])
            nc.scalar.activation(lse_sb[:], ss_sb[:], AF.Ln)
            for t in range(NT):
                vp = LT if t == NT - 1 else PART
                ot = sp.tile([PART, NCLS], F32, tag="es")
                nc.vector.tensor_scalar(ot[0:vp, :], pys[0:vp, t, :],
                                        nm_sb[0:vp, t:t + 1],
                                        lse_sb[0:vp, t:t + 1],
                                        ALU.add, ALU.subtract)
                r0 = t * PART
                nc.sync.dma_start(out[r0:r0 + vp, :], ot[0:vp, :])

    _finalize(nc)
    return nc


# ---------------------------------------------------------------------------
# entry point
# ---------------------------------------------------------------------------

def _in_maps(cfg, g, x, W_in, b_in, W_out, b_out, delta, a):
    ident = np.eye(PART, dtype=np.float32).astype(ml_dtypes.bfloat16)
    maps = []
    for m in range(PCORES):
        maps.append({
            "xs": np.ascontiguousarray(
                x[m * cfg.SHARD:(m + 1) * cfg.SHARD].T).astype(
                    ml_dtypes.bfloat16),
            "Wi": np.ascontiguousarray(W_in).astype(np.float32),
            "bi": b_in.reshape(1, -1).astype(np.float32),
            "Wo": np.ascontiguousarray(W_out).astype(np.float32),
            "bo": b_out.reshape(1, -1).astype(np.float32),
            "dl": delta.reshape(1, 1).astype(np.float32),
            "av": a.reshape(1, 1).astype(np.float32),
            "idt": ident,
            "sblob": g.sblobs[m],
            "iblob": g.iblobs[m],
        })
    return maps


def prepare(x, vals, W_in, b_in, delta, a, W_out, b_out, rows, cols,
            debug=False, **cfg_kw):
    x = np.asarray(x)
    cfg = Cfg(N=x.shape[0], E=len(np.asarray(vals)), CIN=x.shape[1],
              COUT=np.asarray(W_in).shape[1], NCLS=np.asarray(W_out).shape[1],
              **cfg_kw)
    g = build_grid(cfg, np.asarray(rows), np.asarray(cols),
                   np.asarray(vals, np.float32))
    nc = build_nc(cfg, g, debug=debug)
    maps = _in_maps(cfg, g, x, np.asarray(W_in), np.asarray(b_in),
                    np.asarray(W_out), np.asarray(b_out),
                    np.asarray(delta), np.asarray(a))
    return cfg, g, nc, maps


def kernel(x, vals, W_in, b_in, delta, a, W_out, b_out, rows, cols):
    from concourse.bass_utils import run_bass_kernel_spmd

    cfg, g, nc, maps = prepare(x, vals, W_in, b_in, delta, a, W_out, b_out,
                               rows, cols)
    res = run_bass_kernel_spmd(nc, maps, core_ids=list(range(PCORES)))
    return np.concatenate([res.results[m]["out"] for m in range(PCORES)], 0)



# revision 34
# speedup vs baseline: 2.3484x; 1.0781x over previous
"""AnomalyNet (3-hop Chebyshev-style GNN) on 8 Trainium2 NeuronCores.

Strategy:
  - Shard nodes (rows) across 8 cores: core m owns rows [m*SHARD, (m+1)*SHARD).
  - Dense parts (x @ W_in, epilogue @ W_out) on TensorE, bf16.
  - Each spmm hop: the full feature table [N, 128] bf16 lives in HBM
    (AllGather of per-core shard outputs).  Each core gathers its edges'
    source rows with dma_gather (one 256B descriptor per edge), and reduces
    into rows via PE "segment matmuls": for each chunk of 128 edge slots, a
    host-built scatter matrix S [128 slots, 32 window-rows] (vals baked in,
    bf16) is the stationary operand; out[window, :] += S^T @ gathered.
  - int16 gather indices cap the table height at 32768, so gathers read one
    of two overlapping views of the table (rows [0, 32768) / [N-32768, N));
    edges in the overlap are assigned to balance chunk counts.
  - The chunk grid (windows x chunk counts) is shared by all 8 cores (SPMD:
    one instruction stream); per-core edge data (indices, S values) comes in
    via per-core input blobs.  Pad slots point at a real row with val 0.
  - 4th sparse pass (high_delta = L @ dn) reuses the same chunks/indices with
    a table of dn broadcast across 128 channels.
  - Global min/max of high_delta via a 2-element AllReduce(max) of
    [max, -min].
"""

import math
import os

import numpy as np
import ml_dtypes

import concourse.bacc as bacc
import concourse.bass as bass
import concourse.mybir as mybir
import concourse.tile as tile

F32 = mybir.dt.float32
BF16 = mybir.dt.bfloat16
I16 = mybir.dt.int16
AF = mybir.ActivationFunctionType
ALU = mybir.AluOpType
AXL = mybir.AxisListType

PCORES = 8
PART = 128  # partitions


class Cfg:
    def __init__(self, N=50000, E=800000, CIN=256, COUT=128, NCLS=7,
                 W=32, NBB=2, IDX_CAP=32768):
        assert N % PCORES == 0
        self.N, self.E, self.CIN, self.COUT, self.NCLS = N, E, CIN, COUT, NCLS
        self.SHARD = N // PCORES
        self.W = W                      # window rows per chunk
        self.WPB = PART // W            # windows per 128-row block
        self.NT = math.ceil(self.SHARD / PART)   # 128-row blocks per core
        self.LT = self.SHARD - (self.NT - 1) * PART  # rows in last block
        self.NW = math.ceil(self.SHARD / W)      # windows per core
        self.NBB = NBB                  # blocks per gather batch
        self.TA_H = min(N, IDX_CAP)     # table A = rows [0, TA_H)
        self.B_BASE = max(0, N - IDX_CAP)  # table B = rows [B_BASE, N)
        self.CINT = math.ceil(CIN / PART)  # 128-col tiles of CIN
        self.NTP = math.ceil(self.SHARD / 16) * 16  # shard rows padded (xbar)
        # AllGather pieces (block-aligned, small last piece so the only
        # non-overlapped AG is cheap): list of (b0, b1, row_off, rows)
        NP = 4
        nb = [self.NT // NP] * NP
        for i in range(self.NT % NP):
            nb[i] += 1
        if NP > 1 and nb[-1] > 2:       # shrink the exposed tail piece
            nb[-1] -= 2
            nb[0] += 1
            nb[1] += 1
        self.pieces = []
        b0 = 0
        for i, n in enumerate(nb):
            b1 = b0 + n
            off = b0 * PART
            rows = (self.SHARD if b1 == self.NT else b1 * PART) - off
            self.pieces.append((b0, b1, off, rows))
            b0 = b1
        # table-row permutation: piece-major, rank-major inside a piece
        poff = np.zeros(len(self.pieces) + 1, np.int64)
        for i, (_, _, off, rows) in enumerate(self.pieces):
            poff[i + 1] = poff[i] + rows * PCORES
        self.poff = poff
        self.piece_end = {b1 - 1: i for i, (b0, b1, _, _) in
                          enumerate(self.pieces)}
        r = np.arange(N, dtype=np.int64) % self.SHARD
        m = np.arange(N, dtype=np.int64) // self.SHARD
        bounds = np.array([p[2] for p in self.pieces] + [self.SHARD])
        pid = np.searchsorted(bounds, r, side="right") - 1
        rows_of = np.array([p[3] for p in self.pieces])
        offs_of = np.array([p[2] for p in self.pieces])
        self.perm = poff[pid] + m * rows_of[pid] + (r - offs_of[pid])
        assert len(np.unique(self.perm)) == N


# ---------------------------------------------------------------------------
# host-side preprocessing: build the (core-uniform) chunk grid and per-core
# index / S blobs
# ---------------------------------------------------------------------------

class Grid:
    """Uniform structure shared by all cores + per-core data blobs."""
    pass


def build_grid(cfg: Cfg, rows: np.ndarray, cols: np.ndarray, vals: np.ndarray):
    N, W = cfg.N, cfg.W
    SHARD, NW = cfg.SHARD, cfg.NW

    core_of = rows // SHARD
    rloc = rows - core_of * SHARD
    wid = rloc // W

    # table rows are permuted (piece-major AllGather layout)
    cols = cfg.perm[cols]

    # category: 0 = A-only (c < B_BASE), 1 = flex, 2 = B-only (c >= TA_H)
    cat = np.ones(cfg.E, np.int8)
    cat[cols < cfg.B_BASE] = 0
    cat[cols >= cfg.TA_H] = 2

    order = np.lexsort((cat, wid, core_of))
    counts = np.zeros((PCORES, NW, 3), np.int64)
    np.add.at(counts, (core_of, wid, cat), 1)
    starts = np.zeros((PCORES, NW, 3), np.int64)
    starts.reshape(-1)[1:] = np.cumsum(counts.reshape(-1))[:-1]

    nA0 = counts[:, :, 0]
    nF = counts[:, :, 1]
    nB0 = counts[:, :, 2]
    ntot = nA0 + nF + nB0

    # uniform chunk counts per window
    kA = np.ceil(nA0.max(0) / 128).astype(int)
    kB = np.ceil(nB0.max(0) / 128).astype(int)
    need = np.ceil(ntot.max(0) / 128).astype(int)
    for w in range(NW):
        while kA[w] + kB[w] < max(need[w], 1):
            slackA = kA[w] * 128 - nA0[:, w].max()
            slackB = kB[w] * 128 - nB0[:, w].max()
            if kB[w] == 0 or slackA <= slackB:
                kA[w] += 1
            else:
                kB[w] += 1
    assert (kA[None, :] * 128 >= nA0).all()
    assert (kB[None, :] * 128 >= nB0).all()
    assert ((kA + kB)[None, :] * 128 >= ntot).all()

    g = Grid()
    g.kA, g.kB = kA, kB
    g.nchunks = int((kA + kB).sum())

    # chunk gidx base per window (A-chunks then B-chunks, windows in order)
    wbaseA = np.zeros(NW, np.int64)
    wbaseB = np.zeros(NW, np.int64)
    base = 0
    for w in range(NW):
        wbaseA[w] = base
        wbaseB[w] = base + kA[w]
        base += kA[w] + kB[w]

    # batches of NBB blocks; within a batch: chunks ordered by block, window,
    # A-chunks then B-chunks.  Gather call A covers the batch's A-chunks in
    # that order; call B likewise.
    nbatch = math.ceil(cfg.NT / cfg.NBB)
    batches = []
    idx_cols_total = 0
    for bi in range(nbatch):
        blocks = range(bi * cfg.NBB, min((bi + 1) * cfg.NBB, cfg.NT))
        ch = []
        for b in blocks:
            for w in range(b * cfg.WPB, min((b + 1) * cfg.WPB, NW)):
                nw_ch = int(kA[w] + kB[w])
                for i in range(nw_ch):
                    gi = int((wbaseA[w] if i < kA[w] else wbaseB[w] - kA[w])
                             + i)
                    ch.append(dict(g=gi, w=w, tab=0 if i < kA[w] else 1, b=b,
                                   first=(i == 0), last=(i == nw_ch - 1)))
        ncA = sum(1 for c in ch if c["tab"] == 0)
        ncB = len(ch) - ncA
        pA = pB = 0
        for c in ch:
            if c["tab"] == 0:
                c["pos"] = pA
                pA += 1
            else:
                c["pos"] = pB
                pB += 1
        batches.append(dict(blocks=list(blocks), ncA=ncA, ncB=ncB, chunks=ch,
                            idx_colA=idx_cols_total,
                            idx_colB=idx_cols_total + ncA * 8))
        idx_cols_total += (ncA + ncB) * 8
    g.batches = batches
    g.idx_cols_total = idx_cols_total

    # ---- per-core blobs
    g.sblobs = []
    g.iblobs = []
    for m in range(PCORES):
        S = np.zeros((g.nchunks, PART, W), np.float32)
        idx_all = np.zeros((g.nchunks, PART), np.int16)

        for w in range(NW):
            e0a, n0a = starts[m, w, 0], counts[m, w, 0]
            e0f, n0f = starts[m, w, 1], counts[m, w, 1]
            e0b, n0b = starts[m, w, 2], counts[m, w, 2]
            capA = kA[w] * 128
            fA = min(n0f, capA - n0a)
            assert n0b + (n0f - fA) <= kB[w] * 128

            selA = np.concatenate([order[e0a:e0a + n0a], order[e0f:e0f + fA]])
            selB = np.concatenate([order[e0f + fA:e0f + n0f],
                                   order[e0b:e0b + n0b]])
            for sel, k, gbase, off in (
                (selA, int(kA[w]), int(wbaseA[w]), 0),
                (selB, int(kB[w]), int(wbaseB[w]), cfg.B_BASE),
            ):
                ns = len(sel)
                if k == 0:
                    assert ns == 0
                    continue
                ids = (cols[sel] - off).astype(np.int16)
                vs = vals[sel]
                rr = (rloc[sel] - w * W).astype(np.int64)
                cap = k * 128
                pad_idx = ids[-1] if ns else np.int16(0)
                full_ids = np.full(cap, pad_idx, np.int16)
                full_ids[:ns] = ids
                for j in range(k):
                    gi = gbase + j
                    seg = np.arange(j * 128, min((j + 1) * 128, ns))
                    if len(seg):
                        S[gi, seg - j * 128, rr[seg]] = vs[seg]
                    idx_all[gi] = full_ids[j * 128:(j + 1) * 128]

        iblob = np.zeros((16, idx_cols_total), np.int16)
        for bt in batches:
            for c in bt["chunks"]:
                col0 = (bt["idx_colA"] if c["tab"] == 0 else bt["idx_colB"]) \
                    + c["pos"] * 8
                iblob[:, col0:col0 + 8] = idx_all[c["g"]].reshape(8, 16).T
        g.sblobs.append(
            np.ascontiguousarray(S.transpose(1, 0, 2)).astype(
                ml_dtypes.bfloat16))
        g.iblobs.append(iblob)
    return g


# ---------------------------------------------------------------------------
# kernel builder (SPMD graph, shared by all cores)
# ---------------------------------------------------------------------------

def _finalize(nc):
    nc.compile()


# Bacc.compile()'s late passes (library/act-table loads, hostgen rebases) run
# after its last generate_event_semaphores() and can leave DMA instructions
# with 2 sync waits; walrus's DIRECT2D lowering has a single wait slot and
# dies with "Too many sync wait commands".  Splice one more splitter pass in
# right before ISA codegen (the last point where inserted EventSemaphores
# still go through codegen).
import concourse.bacc as _bacc_mod

if not getattr(_bacc_mod.Bacc, "_evsem_patch", False):
    _orig_codegen = _bacc_mod.Bacc.codegen_inst_isa_subclasses

    def _codegen_with_split(self):
        self.generate_event_semaphores()
        return _orig_codegen(self)

    _bacc_mod.Bacc.codegen_inst_isa_subclasses = _codegen_with_split
    _bacc_mod.Bacc._evsem_patch = True


def build_nc(cfg: Cfg, g, debug=False):
    STAGE = int(os.environ.get("KSTAGE", "99"))
    NQ = int(os.environ.get("KQ", "4"))  # SWDGE queues for gather desc-gen
    nc = bacc.Bacc("TRN2", target_bir_lowering=False, debug=debug,
                   num_devices=PCORES, num_swdge_queues=NQ,
                   dynamic_dma_scratch_size=16384)
    N, COUT, NCLS, W = cfg.N, cfg.COUT, cfg.NCLS, cfg.W
    SHARD, NT, LT = cfg.SHARD, cfg.NT, cfg.LT
    RG = [list(range(PCORES))]

    # ---------------- dram parameters
    xs = nc.declare_dram_parameter("xs", [cfg.CIN, SHARD], BF16, isOutput=False)
    Wi = nc.declare_dram_parameter("Wi", [cfg.CIN, COUT], F32, isOutput=False)
    bi = nc.declare_dram_parameter("bi", [1, COUT], F32, isOutput=False)
    Wo = nc.declare_dram_parameter("Wo", [COUT, NCLS], F32, isOutput=False)
    bo = nc.declare_dram_parameter("bo", [1, NCLS], F32, isOutput=False)
    dl = nc.declare_dram_parameter("dl", [1, 1], F32, isOutput=False)
    av = nc.declare_dram_parameter("av", [1, 1], F32, isOutput=False)
    idt = nc.declare_dram_parameter("idt", [PART, PART], BF16, isOutput=False)
    sbl = nc.declare_dram_parameter("sblob", [PART, g.nchunks, W], BF16,
                                    isOutput=False)
    ibl = nc.declare_dram_parameter("iblob", [16, g.idx_cols_total], I16,
                                    isOutput=False)
    out = nc.declare_dram_parameter("out", [SHARD, NCLS], F32, isOutput=True)

    # ---------------- internal dram
    agin = [nc.dram_tensor(f"agin{k}", [SHARD, COUT], BF16) for k in range(4)]
    tbl = [nc.dram_tensor(f"tbl{k}", [N, COUT], BF16, addr_space="Shared")
           for k in range(4)]
    mm_in = nc.dram_tensor("mm_in", [1, 2], F32)
    mm_out = nc.dram_tensor("mm_out", [1, 2], F32, addr_space="Shared")

    def store_piece(k, sb3, pi):
        """Store piece pi's blocks of sb3 [128, NT, COUT] into agin[k]."""
        b0, b1, off, rows = cfg.pieces[pi]
        full = b1 - b0 - (1 if b1 == NT else 0)
        if full:
            nc.sync.dma_start(
                agin[k][off:off + full * PART, :].rearrange(
                    "(t p) c -> p t c", p=PART),
                sb3[:, b0:b0 + full, :])
        if b1 == NT:
            nc.sync.dma_start(agin[k][off + full * PART:off + rows, :],
                              sb3[0:LT, NT - 1, :])

    def ag_piece(k, pi):
        """AllGather piece pi of agin[k] into the piece-major tbl[k]."""
        _, _, off, rows = cfg.pieces[pi]
        t0 = int(cfg.poff[pi])
        nc.gpsimd.collective_compute(
            "AllGather", ALU.bypass, RG,
            ins=[agin[k][off:off + rows, :]],
            outs=[tbl[k][t0:t0 + PCORES * rows, :]])

    def transpose(out_ap, in_ap, ident, k):
        nc.tensor.matmul(out_ap, in_ap, ident[0:k, 0:k], is_transpose=True,
                         start=True, stop=True, skip_group_check=True)

    with tile.TileContext(nc) as tc:
        with (
            tc.tile_pool(name="const", bufs=1) as cp,
            tc.tile_pool(name="resid", bufs=1) as rp,
            tc.tile_pool(name="gat", bufs=2 * NQ) as gp,
            tc.tile_pool(name="stage", bufs=3) as sp,
            tc.tile_pool(name="pbig", bufs=4, space="PSUM") as pb,
            tc.tile_pool(name="psmall", bufs=2, space="PSUM") as ps,
            tc.tile_pool(name="phd", bufs=2, space="PSUM") as ph,
        ):
            # ---------------- constants
            Wi_sb = cp.tile([PART, cfg.CINT, COUT], BF16)
            for h in range(cfg.CINT):
                hi = min(cfg.CIN - h * PART, PART)
                nc.gpsimd.dma_start(Wi_sb[0:hi, h, :],
                                    Wi[h * PART:h * PART + hi, :])
            bi_sb = cp.tile([1, COUT], BF16)
            nc.gpsimd.dma_start(bi_sb[:], bi[:])
            Wo_sb = cp.tile([PART, NCLS], BF16)
            nc.gpsimd.dma_start(Wo_sb[0:COUT, :], Wo[:])
            bo_sb = cp.tile([1, NCLS], BF16)
            nc.gpsimd.dma_start(bo_sb[:], bo[:])
            id_bf = cp.tile([PART, PART], BF16)
            nc.sync.dma_start(id_bf[:], idt[:])
            id_f32 = cp.tile([PART, PART], F32)
            nc.vector.tensor_copy(id_f32[:], id_bf[:])
            ones1b = cp.tile([1, PART], BF16)
            nc.vector.memset(ones1b[:], 1.0)
            ones1f = cp.tile([1, PART], F32)
            nc.vector.memset(ones1f[:], 1.0)
            onesCb = cp.tile([PART, COUT], BF16)
            nc.vector.memset(onesCb[:], 1.0)
            dl_sb = cp.tile([1, 1], F32)
            nc.sync.dma_start(dl_sb[:], dl[:])
            av_sb = cp.tile([1, 1], F32)
            nc.sync.dma_start(av_sb[:], av[:])

            idx_sb = cp.tile([PART, g.idx_cols_total], I16)
            for k in range(8):
                nc.sync.dma_start(idx_sb[16 * k:16 * (k + 1), :], ibl[:])
            s_sb = cp.tile([PART, g.nchunks, W], BF16)
            nc.sync.dma_start(s_sb[:], sbl[:])

            # resident feature tiles [128, NT, COUT]
            h_sb = rp.tile([PART, NT, COUT], BF16)
            t1_sb = rp.tile([PART, NT, COUT], BF16)
            t2_sb = rp.tile([PART, NT, COUT], BF16)
            u_sb = rp.tile([PART, NT, COUT], BF16)
            dn_sb = rp.tile([PART, NT], F32)
            nc.vector.memset(dn_sb[:], 0.0)
            hd_sb = rp.tile([PART, NT], F32)
            ns_sb = rp.tile([PART, NT], F32)

            # ---------------- coefficients (f32 [1,1] lane ops)
            cofp = cp.tile([1, 8], F32)  # c2L c1L c0L1 n2d d2
            nc.vector.memset(cofp[:], 0.0)
            t_ = cp.tile([1, 6], F32)
            d_, a_ = dl_sb[:], av_sb[:]
            d2_ = t_[:, 0:1]
            nc.vector.tensor_mul(d2_, d_, d_)
            d3_ = t_[:, 1:2]
            nc.vector.tensor_mul(d3_, d2_, d_)
            da_ = t_[:, 2:3]
            nc.vector.tensor_mul(da_, d_, a_)
            d2a_ = t_[:, 3:4]
            nc.vector.tensor_mul(d2a_, d2_, a_)
            nc.vector.scalar_tensor_tensor(cofp[:, 0:1], d_, -3.0, a_,
                                           ALU.mult, ALU.subtract)  # -3d - a
            t30 = t_[:, 4:5]
            nc.vector.tensor_scalar_mul(t30, d2_, 3.0)
            nc.vector.scalar_tensor_tensor(cofp[:, 1:2], da_, 2.0, t30,
                                           ALU.mult, ALU.add)  # 3d2 + 2da
            t31 = t_[:, 5:6]
            nc.vector.tensor_add(t31, d3_, d2a_)
            nc.vector.tensor_scalar(cofp[:, 2:3], t31, -1.0, -1.0,
                                    ALU.mult, ALU.add)  # -(d3 + d2a) - 1
            nc.vector.tensor_scalar_mul(cofp[:, 3:4], d_, -2.0)  # -2d
            nc.vector.tensor_copy(cofp[:, 4:5], d2_)  # d2
            pco = ps.tile([PART, 8], F32, tag="sm")
            nc.tensor.matmul(pco[:], ones1f[:], cofp[:], start=True, stop=True)
            cf_sb = cp.tile([PART, 8], F32)
            nc.scalar.copy(cf_sb[:], pco[:])
            C2L, C1L, C0L1 = cf_sb[:, 0:1], cf_sb[:, 1:2], cf_sb[:, 2:3]
            N2D, D2C = cf_sb[:, 3:4], cf_sb[:, 4:5]

            # ---------------- prologue: h = tanh(x @ Wi + bi)
            with tc.tile_pool(name="xt", bufs=3) as xp:
                for t in range(NT):
                    vp = LT if t == NT - 1 else PART
                    r0 = t * PART
                    xTt = xp.tile([PART, cfg.CINT, PART], BF16,
                                  name="xTt", tag="xTt")
                    for h in range(cfg.CINT):
                        hi = min(cfg.CIN - h * PART, PART)
                        nc.sync.dma_start(
                            xTt[0:hi, h, 0:vp],
                            xs[h * PART:h * PART + hi, r0:r0 + vp])
                    pt = pb.tile([PART, COUT], F32, tag="big")
                    for h in range(cfg.CINT):
                        nc.tensor.matmul(pt[0:vp, :], xTt[:, h, 0:vp],
                                         Wi_sb[:, h, :], start=(h == 0),
                                         stop=False)
                    nc.tensor.matmul(pt[0:vp, :], ones1b[:, 0:vp], bi_sb[:],
                                     start=False, stop=True)
                    nc.scalar.activation(h_sb[0:vp, t, :], pt[0:vp, :],
                                         AF.Tanh)
                    if t in cfg.piece_end:
                        pi = cfg.piece_end[t]
                        store_piece(0, h_sb, pi)
                        ag_piece(0, pi)

            def _early_out():
                for t in range(NT):
                    vp = LT if t == NT - 1 else PART
                    zt = sp.tile([PART, NCLS], F32, tag="es", name="zt")
                    nc.vector.memset(zt[:], 0.0)
                    nc.gpsimd.dma_start(out[t * PART:t * PART + vp, :],
                                        zt[0:vp, :])

            # ---------------- generic sparse hop
            def hop(tbl_in, nfree, on_block_done):
                tA = tbl_in[0:cfg.TA_H, :]
                tB = tbl_in[cfg.B_BASE:N, :]
                for bi, bt in enumerate(g.batches):
                    q = bi % NQ
                    gA = gB = None
                    if bt["ncA"]:
                        gA = gp.tile([PART, bt["ncA"], COUT], BF16, tag="g")
                        nc.gpsimd.dma_gather(
                            gA[:], tA,
                            idx_sb[:, bt["idx_colA"]:
                                   bt["idx_colA"] + bt["ncA"] * 8],
                            bt["ncA"] * PART, bt["ncA"] * PART, COUT,
                            single_packet=False, queue_num=q)
                    if bt["ncB"]:
                        gB = gp.tile([PART, bt["ncB"], COUT], BF16, tag="g")
                        nc.gpsimd.dma_gather(
                            gB[:], tB,
                            idx_sb[:, bt["idx_colB"]:
                                   bt["idx_colB"] + bt["ncB"] * 8],
                            bt["ncB"] * PART, bt["ncB"] * PART, COUT,
                            single_packet=False, queue_num=q)
                    cur_b = -1
                    psum = None
                    for c in bt["chunks"]:
                        if c["b"] != cur_b:
                            if cur_b >= 0:
                                on_block_done(cur_b, psum)
                            cur_b = c["b"]
                            if nfree == COUT:
                                psum = pb.tile([PART, COUT], F32, tag="big",
                                               name="psum_hop")
                            else:
                                psum = ph.tile([PART, 1], F32, tag="hd",
                                               name="psum_hd")
                        woff = (c["w"] % cfg.WPB) * W
                        gt = gA if c["tab"] == 0 else gB
                        nc.tensor.matmul(
                            psum[woff:woff + W, 0:nfree],
                            s_sb[:, c["g"], :],
                            gt[:, c["pos"], 0:nfree],
                            start=c["first"], stop=c["last"],
                            skip_group_check=True,
                            tile_position=(0, woff))
                    if cur_b >= 0:
                        on_block_done(cur_b, psum)

            if STAGE < 1:
                _early_out()
                _finalize(nc)
                return nc

            # hop 1: T1
            def done1(b, psum):
                vp = LT if b == NT - 1 else PART
                nc.scalar.copy(t1_sb[0:vp, b, :], psum[0:vp, :])
                if b in cfg.piece_end:
                    pi = cfg.piece_end[b]
                    store_piece(1, t1_sb, pi)
                    ag_piece(1, pi)
            hop(tbl[0], COUT, done1)

            if STAGE < 2:
                _early_out()
                _finalize(nc)
                return nc

            # hop 2: T2 (db computed later, in hop 3's callback)
            def done2(b, psum):
                vp = LT if b == NT - 1 else PART
                nc.scalar.copy(t2_sb[0:vp, b, :], psum[0:vp, :])
                if b in cfg.piece_end:
                    pi = cfg.piece_end[b]
                    store_piece(2, t2_sb, pi)
                    ag_piece(2, pi)
            hop(tbl[1], COUT, done2)

            if STAGE < 3:
                _early_out()
                _finalize(nc)
                return nc

            # hop 3: u = T3 + c2L*T2 + c1L*T1 + (c0L-1)*h ; dn2 = row sumsq
            def done3(b, psum):
                vp = LT if b == NT - 1 else PART
                e1 = sp.tile([PART, COUT], F32, tag="e1")
                nc.vector.scalar_tensor_tensor(
                    e1[0:vp, :], t2_sb[0:vp, b, :], C2L[0:vp, :],
                    psum[0:vp, :], ALU.mult, ALU.add)
                nc.vector.scalar_tensor_tensor(
                    e1[0:vp, :], t1_sb[0:vp, b, :], C1L[0:vp, :],
                    e1[0:vp, :], ALU.mult, ALU.add)
                nc.vector.scalar_tensor_tensor(
                    u_sb[0:vp, b, :], h_sb[0:vp, b, :], C0L1[0:vp, :],
                    e1[0:vp, :], ALU.mult, ALU.add)
                sq = sp.tile([PART, COUT], F32, tag="sq")
                nc.scalar.activation(sq[0:vp, :], u_sb[0:vp, b, :], AF.Square,
                                     accum_out=dn_sb[0:vp, b:b + 1])
                # t2 is no longer needed as-is: overwrite with
                # db = T2 - 2d*T1 + d2*h for the epilogue
                nc.vector.scalar_tensor_tensor(
                    t2_sb[0:vp, b, :], t1_sb[0:vp, b, :], N2D[0:vp, :],
                    t2_sb[0:vp, b, :], ALU.mult, ALU.add)
                nc.vector.scalar_tensor_tensor(
                    t2_sb[0:vp, b, :], h_sb[0:vp, b, :], D2C[0:vp, :],
                    t2_sb[0:vp, b, :], ALU.mult, ALU.add)
                if b in cfg.piece_end:
                    # dn = sqrt(sumsq) for this piece's blocks, broadcast
                    # across channels into agin[3], AllGather into tbl[3]
                    pi = cfg.piece_end[b]
                    b0, b1, off, rows = cfg.pieces[pi]
                    nc.scalar.sqrt(dn_sb[:, b0:b1], dn_sb[:, b0:b1])
                    for bb in range(b0, b1):
                        vpb = LT if bb == NT - 1 else PART
                        dnb_t = sp.tile([PART, COUT], BF16, tag="fr")
                        nc.vector.tensor_scalar_mul(dnb_t[0:vpb, :],
                                                    onesCb[0:vpb, :],
                                                    dn_sb[0:vpb, bb:bb + 1])
                        nc.sync.dma_start(
                            agin[3][bb * PART:bb * PART + vpb, :],
                            dnb_t[0:vpb, :])
                    ag_piece(3, pi)
            hop(tbl[2], COUT, done3)

            if STAGE < 4:
                _early_out()
                _finalize(nc)
                return nc

            if STAGE < 5:
                _early_out()
                _finalize(nc)
                return nc

            # hop 4: hd = L @ dn (vector is idle here: fold u <- u + h
            # so the epilogue's final mix is a single op per tile)
            def done4(b, psum):
                vp = LT if b == NT - 1 else PART
                nc.scalar.copy(hd_sb[0:vp, b:b + 1], psum[0:vp, 0:1])
                nc.vector.tensor_add(u_sb[0:vp, b, :], u_sb[0:vp, b, :],
                                     h_sb[0:vp, b, :])
            hop(tbl[3], 1, done4)

            if STAGE < 6:
                _early_out()
                _finalize(nc)
                return nc

            # ---------------- global min/max -> AllReduce(max) of [max, -min]
            mx1 = sp.tile([PART, 1], F32, tag="mm")
            mn1 = sp.tile([PART, 1], F32, tag="mm")
            if NT > 1:
                nc.vector.reduce_max(mx1[:], hd_sb[:, 0:NT - 1], axis=AXL.X)
                nc.vector.tensor_reduce(mn1[:], hd_sb[:, 0:NT - 1], axis=AXL.X, op=ALU.min)
                nc.vector.tensor_tensor(mx1[0:LT, :], mx1[0:LT, :],
                                        hd_sb[0:LT, NT - 1:NT], op=ALU.max)
                nc.vector.tensor_tensor(mn1[0:LT, :], mn1[0:LT, :],
                                        hd_sb[0:LT, NT - 1:NT], op=ALU.min)
            else:
                nc.vector.reduce_max(mx1[0:LT, :], hd_sb[0:LT, :], axis=AXL.X)
                nc.vector.tensor_reduce(mn1[0:LT, :], hd_sb[0:LT, :],
                                        axis=AXL.X, op=ALU.min)
            nc.vector.tensor_scalar_mul(mn1[:], mn1[:], -1.0)
            pmx = ps.tile([PART, PART], F32, tag="sm")
            transpose(pmx[0:1, 0:PART], mx1[:], id_f32, PART)
            pmn = ps.tile([PART, PART], F32, tag="sm")
            transpose(pmn[0:1, 0:PART], mn1[:], id_f32, PART)
            mm_sb = sp.tile([1, 2], F32, tag="mm2")
            nc.vector.reduce_max(mm_sb[0:1, 0:1], pmx[0:1, 0:PART], axis=AXL.X)
            nc.vector.reduce_max(mm_sb[0:1, 1:2], pmn[0:1, 0:PART], axis=AXL.X)
            nc.sync.dma_start(mm_in[:], mm_sb[:])
            nc.gpsimd.collective_compute("AllReduce", ALU.max, RG,
                                         ins=[mm_in[:]], outs=[mm_out[:]])
            mmg = sp.tile([1, 2], F32, tag="mm2")
            nc.sync.dma_start(mmg[:], mm_out[:])

            # s = 2a/(mx - mn); ns = (hd + (-mn)) * s  (= normal * 2a)
            sc = cp.tile([1, 2], F32)
            nc.vector.tensor_add(sc[:, 0:1], mmg[:, 0:1], mmg[:, 1:2])
            nc.vector.reciprocal(sc[:, 0:1], sc[:, 0:1])
            nc.vector.tensor_mul(sc[:, 0:1], sc[:, 0:1], av_sb[:])
            nc.vector.tensor_scalar_mul(sc[:, 0:1], sc[:, 0:1], 2.0)
            nc.vector.tensor_copy(sc[:, 1:2], mmg[:, 1:2])
            pbc = ps.tile([PART, PART], F32, tag="sm")
            nc.tensor.matmul(pbc[0:PART, 0:2], ones1f[:], sc[:],
                             start=True, stop=True, skip_group_check=True)
            bc = cp.tile([PART, 2], F32)
            nc.scalar.copy(bc[:], pbc[0:PART, 0:2])
            nc.vector.tensor_scalar(ns_sb[:], hd_sb[:], bc[:, 1:2],
                                    bc[:, 0:1], ALU.add, ALU.mult)

            if STAGE < 7:
                _early_out()
                _finalize(nc)
                return nc

            # ---------------- epilogue (phase-split to avoid act-table thrash)
            pys = rp.tile([PART, NT, NCLS], F32)
            nm_sb = rp.tile([PART, NT], F32)
            ss_sb = rp.tile([PART, NT], F32)
            lse_sb = rp.tile([PART, NT], F32)
            nc.vector.memset(ss_sb[:], 1.0)
            for t in range(NT):
                vp = LT if t == NT - 1 else PART
                # t2_sb holds db, u_sb holds u + h (precomputed in hops 3/4)
                f1 = sp.tile([PART, COUT], F32, tag="f1")
                nc.vector.scalar_tensor_tensor(
                    f1[0:vp, :], t2_sb[0:vp, t, :], ns_sb[0:vp, t:t + 1],
                    u_sb[0:vp, t, :], ALU.mult, ALU.add)
                fr = sp.tile([PART, COUT], BF16, tag="fr")
                nc.scalar.activation(fr[0:vp, :], f1[0:vp, :], AF.Relu)
                ptr = pb.tile([PART, PART], BF16, tag="big")
                transpose(ptr[0:COUT, 0:vp], fr[0:vp, :], id_bf, vp)
                frT = sp.tile([PART, PART], BF16, tag="frT")
                nc.vector.tensor_copy(frT[0:COUT, 0:vp], ptr[0:COUT, 0:vp])
                py = ps.tile([PART, NCLS], F32, tag="sm")
                nc.tensor.matmul(py[0:vp, :], frT[:, 0:vp], Wo_sb[:],
                                 start=True, stop=False)
                nc.tensor.matmul(py[0:vp, :], ones1b[:, 0:vp], bo_sb[:],
                                 start=False, stop=True)
                nc.vector.tensor_copy(pys[0:vp, t, :], py[0:vp, :])
                nc.vector.reduce_max(nm_sb[0:vp, t:t + 1], py[0:vp, :],
                                     axis=AXL.X, negate=True)
            for t in range(NT):
                vp = LT if t == NT - 1 else PART
                es = sp.tile([PART, NCLS], F32, tag="es")
                nc.scalar.activation(es[0:vp, :], pys[0:vp, t, :], AF.Exp,
                                     bias=nm_sb[0:vp, t:t + 1],
                                     accum_out=ss_sb[0:vp, t:t + 1])
            nc.scalar.activation(lse_sb[:], ss_sb[:], AF.Ln)
            for t in range(NT):
                vp = LT if t == NT - 1 else PART
                ot = sp.tile([PART, NCLS], F32, tag="es")
                nc.vector.tensor_scalar(ot[0:vp, :], pys[0:vp, t, :],
                                        nm_sb[0:vp, t:t + 1],
                                        lse_sb[0:vp, t:t + 1],
                                        ALU.add, ALU.subtract)
                r0 = t * PART
                nc.sync.dma_start(out[r0:r0 + vp, :], ot[0:vp, :])

    _finalize(nc)
    return nc


# ---------------------------------------------------------------------------
# entry point
# ---------------------------------------------------------------------------

def _in_maps(cfg, g, x, W_in, b_in, W_out, b_out, delta, a):
    ident = np.eye(PART, dtype=np.float32).astype(ml_dtypes.bfloat16)
    maps = []
    for m in range(PCORES):
        maps.append({
            "xs": np.ascontiguousarray(
                x[m * cfg.SHARD:(m + 1) * cfg.SHARD].T).astype(
                    ml_dtypes.bfloat16),
            "Wi": np.ascontiguousarray(W_in).astype(np.float32),
            "bi": b_in.reshape(1, -1).astype(np.float32),
            "Wo": np.ascontiguousarray(W_out).astype(np.float32),
            "bo": b_out.reshape(1, -1).astype(np.float32),
            "dl": delta.reshape(1, 1).astype(np.float32),
            "av": a.reshape(1, 1).astype(np.float32),
            "idt": ident,
            "sblob": g.sblobs[m],
            "iblob": g.iblobs[m],
        })
    return maps


def prepare(x, vals, W_in, b_in, delta, a, W_out, b_out, rows, cols,
            debug=False, **cfg_kw):
    x = np.asarray(x)
    cfg = Cfg(N=x.shape[0], E=len(np.asarray(vals)), CIN=x.shape[1],
              COUT=np.asarray(W_in).shape[1], NCLS=np.asarray(W_out).shape[1],
              **cfg_kw)
    g = build_grid(cfg, np.asarray(rows), np.asarray(cols),
                   np.asarray(vals, np.float32))
    nc = build_nc(cfg, g, debug=debug)
    maps = _in_maps(cfg, g, x, np.asarray(W_in), np.asarray(b_in),
                    np.asarray(W_out), np.asarray(b_out),
                    np.asarray(delta), np.asarray(a))
    return cfg, g, nc, maps


def kernel(x, vals, W_in, b_in, delta, a, W_out, b_out, rows, cols):
    from concourse.bass_utils import run_bass_kernel_spmd

    cfg, g, nc, maps = prepare(x, vals, W_in, b_in, delta, a, W_out, b_out,
                               rows, cols)
    res = run_bass_kernel_spmd(nc, maps, core_ids=list(range(PCORES)))
    return np.concatenate([res.results[m]["out"] for m in range(PCORES)], 0)



# revision 36
# speedup vs baseline: 2.7042x; 1.1515x over previous
"""AnomalyNet (3-hop Chebyshev-style GNN) on 8 Trainium2 NeuronCores.

Strategy:
  - Shard nodes (rows) across 8 cores: core m owns rows [m*SHARD, (m+1)*SHARD).
  - Dense parts (x @ W_in, epilogue @ W_out) on TensorE, bf16.
  - Each spmm hop: the full feature table [N, 128] bf16 lives in HBM
    (AllGather of per-core shard outputs).  Each core gathers its edges'
    source rows with dma_gather (one 256B descriptor per edge), and reduces
    into rows via PE "segment matmuls": for each chunk of 128 edge slots, a
    host-built scatter matrix S [128 slots, 32 window-rows] (vals baked in,
    bf16) is the stationary operand; out[window, :] += S^T @ gathered.
  - int16 gather indices cap the table height at 32768, so gathers read one
    of two overlapping views of the table (rows [0, 32768) / [N-32768, N));
    edges in the overlap are assigned to balance chunk counts.
  - The chunk grid (windows x chunk counts) is shared by all 8 cores (SPMD:
    one instruction stream); per-core edge data (indices, S values) comes in
    via per-core input blobs.  Pad slots point at a real row with val 0.
  - 4th sparse pass (high_delta = L @ dn) reuses the same chunks/indices with
    a table of dn broadcast across 128 channels.
  - Global min/max of high_delta via a 2-element AllReduce(max) of
    [max, -min].
"""

import math
import os

import numpy as np
import ml_dtypes

import concourse.bacc as bacc
import concourse.bass as bass
import concourse.mybir as mybir
import concourse.tile as tile

F32 = mybir.dt.float32
BF16 = mybir.dt.bfloat16
I16 = mybir.dt.int16
AF = mybir.ActivationFunctionType
ALU = mybir.AluOpType
AXL = mybir.AxisListType

PCORES = 8
PART = 128  # partitions


class Cfg:
    def __init__(self, N=50000, E=800000, CIN=256, COUT=128, NCLS=7,
                 W=32, NBB=2, IDX_CAP=32768):
        assert N % PCORES == 0
        self.N, self.E, self.CIN, self.COUT, self.NCLS = N, E, CIN, COUT, NCLS
        self.SHARD = N // PCORES
        self.W = W                      # window rows per chunk
        self.WPB = PART // W            # windows per 128-row block
        self.NT = math.ceil(self.SHARD / PART)   # 128-row blocks per core
        self.LT = self.SHARD - (self.NT - 1) * PART  # rows in last block
        self.NW = math.ceil(self.SHARD / W)      # windows per core
        self.NBB = NBB                  # blocks per gather batch
        self.TA_H = min(N, IDX_CAP)     # table A = rows [0, TA_H)
        self.B_BASE = max(0, N - IDX_CAP)  # table B = rows [B_BASE, N)
        self.CINT = math.ceil(CIN / PART)  # 128-col tiles of CIN
        self.NTP = math.ceil(self.SHARD / 16) * 16  # shard rows padded (xbar)
        # AllGather pieces (block-aligned, small last piece so the only
        # non-overlapped AG is cheap): list of (b0, b1, row_off, rows)
        NP = 4
        nb = [self.NT // NP] * NP
        for i in range(self.NT % NP):
            nb[i] += 1
        if NP > 1 and nb[-1] > 2:       # shrink the exposed tail piece
            nb[-1] -= 2
            nb[0] += 1
            nb[1] += 1
        self.pieces = []
        b0 = 0
        for i, n in enumerate(nb):
            b1 = b0 + n
            off = b0 * PART
            rows = (self.SHARD if b1 == self.NT else b1 * PART) - off
            self.pieces.append((b0, b1, off, rows))
            b0 = b1
        # table-row permutation: piece-major, rank-major inside a piece
        poff = np.zeros(len(self.pieces) + 1, np.int64)
        for i, (_, _, off, rows) in enumerate(self.pieces):
            poff[i + 1] = poff[i] + rows * PCORES
        self.poff = poff
        self.piece_end = {b1 - 1: i for i, (b0, b1, _, _) in
                          enumerate(self.pieces)}
        r = np.arange(N, dtype=np.int64) % self.SHARD
        m = np.arange(N, dtype=np.int64) // self.SHARD
        bounds = np.array([p[2] for p in self.pieces] + [self.SHARD])
        pid = np.searchsorted(bounds, r, side="right") - 1
        rows_of = np.array([p[3] for p in self.pieces])
        offs_of = np.array([p[2] for p in self.pieces])
        self.perm = poff[pid] + m * rows_of[pid] + (r - offs_of[pid])
        assert len(np.unique(self.perm)) == N


# ---------------------------------------------------------------------------
# host-side preprocessing: build the (core-uniform) chunk grid and per-core
# index / S blobs
# ---------------------------------------------------------------------------

class Grid:
    """Uniform structure shared by all cores + per-core data blobs."""
    pass


def build_grid(cfg: Cfg, rows: np.ndarray, cols: np.ndarray, vals: np.ndarray):
    N, W = cfg.N, cfg.W
    SHARD, NW = cfg.SHARD, cfg.NW

    core_of = rows // SHARD
    rloc = rows - core_of * SHARD
    wid = rloc // W

    # table rows are permuted (piece-major AllGather layout)
    cols = cfg.perm[cols]

    # category: 0 = A-only (c < B_BASE), 1 = flex, 2 = B-only (c >= TA_H)
    cat = np.ones(cfg.E, np.int8)
    cat[cols < cfg.B_BASE] = 0
    cat[cols >= cfg.TA_H] = 2

    order = np.lexsort((cat, wid, core_of))
    counts = np.zeros((PCORES, NW, 3), np.int64)
    np.add.at(counts, (core_of, wid, cat), 1)
    starts = np.zeros((PCORES, NW, 3), np.int64)
    starts.reshape(-1)[1:] = np.cumsum(counts.reshape(-1))[:-1]

    nA0 = counts[:, :, 0]
    nF = counts[:, :, 1]
    nB0 = counts[:, :, 2]
    ntot = nA0 + nF + nB0

    # uniform chunk counts per window
    kA = np.ceil(nA0.max(0) / 128).astype(int)
    kB = np.ceil(nB0.max(0) / 128).astype(int)
    need = np.ceil(ntot.max(0) / 128).astype(int)
    for w in range(NW):
        while kA[w] + kB[w] < max(need[w], 1):
            slackA = kA[w] * 128 - nA0[:, w].max()
            slackB = kB[w] * 128 - nB0[:, w].max()
            if kB[w] == 0 or slackA <= slackB:
                kA[w] += 1
            else:
                kB[w] += 1
    assert (kA[None, :] * 128 >= nA0).all()
    assert (kB[None, :] * 128 >= nB0).all()
    assert ((kA + kB)[None, :] * 128 >= ntot).all()

    g = Grid()
    g.kA, g.kB = kA, kB
    g.nchunks = int((kA + kB).sum())

    # chunk gidx base per window (A-chunks then B-chunks, windows in order)
    wbaseA = np.zeros(NW, np.int64)
    wbaseB = np.zeros(NW, np.int64)
    base = 0
    for w in range(NW):
        wbaseA[w] = base
        wbaseB[w] = base + kA[w]
        base += kA[w] + kB[w]

    # batches of NBB blocks; within a batch: chunks ordered by block, window,
    # A-chunks then B-chunks.  Gather call A covers the batch's A-chunks in
    # that order; call B likewise.
    nbatch = math.ceil(cfg.NT / cfg.NBB)
    batches = []
    idx_cols_total = 0
    for bi in range(nbatch):
        blocks = range(bi * cfg.NBB, min((bi + 1) * cfg.NBB, cfg.NT))
        ch = []
        for b in blocks:
            for w in range(b * cfg.WPB, min((b + 1) * cfg.WPB, NW)):
                nw_ch = int(kA[w] + kB[w])
                for i in range(nw_ch):
                    gi = int((wbaseA[w] if i < kA[w] else wbaseB[w] - kA[w])
                             + i)
                    ch.append(dict(g=gi, w=w, tab=0 if i < kA[w] else 1, b=b,
                                   first=(i == 0), last=(i == nw_ch - 1)))
        ncA = sum(1 for c in ch if c["tab"] == 0)
        ncB = len(ch) - ncA
        pA = pB = 0
        for c in ch:
            if c["tab"] == 0:
                c["pos"] = pA
                pA += 1
            else:
                c["pos"] = pB
                pB += 1
        batches.append(dict(blocks=list(blocks), ncA=ncA, ncB=ncB, chunks=ch,
                            idx_colA=idx_cols_total,
                            idx_colB=idx_cols_total + ncA * 8))
        idx_cols_total += (ncA + ncB) * 8
    g.batches = batches
    g.idx_cols_total = idx_cols_total

    # ---- per-core blobs
    g.sblobs = []
    g.iblobs = []
    for m in range(PCORES):
        S = np.zeros((g.nchunks, PART, W), np.float32)
        idx_all = np.zeros((g.nchunks, PART), np.int16)

        for w in range(NW):
            e0a, n0a = starts[m, w, 0], counts[m, w, 0]
            e0f, n0f = starts[m, w, 1], counts[m, w, 1]
            e0b, n0b = starts[m, w, 2], counts[m, w, 2]
            capA = kA[w] * 128
            fA = min(n0f, capA - n0a)
            assert n0b + (n0f - fA) <= kB[w] * 128

            selA = np.concatenate([order[e0a:e0a + n0a], order[e0f:e0f + fA]])
            selB = np.concatenate([order[e0f + fA:e0f + n0f],
                                   order[e0b:e0b + n0b]])
            for sel, k, gbase, off in (
                (selA, int(kA[w]), int(wbaseA[w]), 0),
                (selB, int(kB[w]), int(wbaseB[w]), cfg.B_BASE),
            ):
                ns = len(sel)
                if k == 0:
                    assert ns == 0
                    continue
                ids = (cols[sel] - off).astype(np.int16)
                vs = vals[sel]
                rr = (rloc[sel] - w * W).astype(np.int64)
                cap = k * 128
                pad_idx = ids[-1] if ns else np.int16(0)
                full_ids = np.full(cap, pad_idx, np.int16)
                full_ids[:ns] = ids
                for j in range(k):
                    gi = gbase + j
                    seg = np.arange(j * 128, min((j + 1) * 128, ns))
                    if len(seg):
                        S[gi, seg - j * 128, rr[seg]] = vs[seg]
                    idx_all[gi] = full_ids[j * 128:(j + 1) * 128]

        iblob = np.zeros((16, idx_cols_total), np.int16)
        for bt in batches:
            for c in bt["chunks"]:
                col0 = (bt["idx_colA"] if c["tab"] == 0 else bt["idx_colB"]) \
                    + c["pos"] * 8
                iblob[:, col0:col0 + 8] = idx_all[c["g"]].reshape(8, 16).T
        g.sblobs.append(
            np.ascontiguousarray(S.transpose(1, 0, 2)).astype(
                ml_dtypes.bfloat16))
        g.iblobs.append(iblob)
    return g


# ---------------------------------------------------------------------------
# kernel builder (SPMD graph, shared by all cores)
# ---------------------------------------------------------------------------

def _finalize(nc):
    nc.compile()


# Bacc.compile()'s late passes (library/act-table loads, hostgen rebases) run
# after its last generate_event_semaphores() and can leave DMA instructions
# with 2 sync waits; walrus's DIRECT2D lowering has a single wait slot and
# dies with "Too many sync wait commands".  Splice one more splitter pass in
# right before ISA codegen (the last point where inserted EventSemaphores
# still go through codegen).
import concourse.bacc as _bacc_mod

if not getattr(_bacc_mod.Bacc, "_evsem_patch", False):
    _orig_codegen = _bacc_mod.Bacc.codegen_inst_isa_subclasses

    def _codegen_with_split(self):
        self.generate_event_semaphores()
        return _orig_codegen(self)

    _bacc_mod.Bacc.codegen_inst_isa_subclasses = _codegen_with_split
    _bacc_mod.Bacc._evsem_patch = True


def build_nc(cfg: Cfg, g, debug=False):
    STAGE = int(os.environ.get("KSTAGE", "99"))
    NQ = int(os.environ.get("KQ", "4"))  # SWDGE queues for gather desc-gen
    nc = bacc.Bacc("TRN2", target_bir_lowering=False, debug=debug,
                   num_devices=PCORES, num_swdge_queues=NQ,
                   dynamic_dma_scratch_size=16384)
    N, COUT, NCLS, W = cfg.N, cfg.COUT, cfg.NCLS, cfg.W
    SHARD, NT, LT = cfg.SHARD, cfg.NT, cfg.LT
    RG = [list(range(PCORES))]

    # ---------------- dram parameters
    xs = nc.declare_dram_parameter("xs", [cfg.CIN, SHARD], BF16, isOutput=False)
    Wi = nc.declare_dram_parameter("Wi", [cfg.CIN, COUT], F32, isOutput=False)
    bi = nc.declare_dram_parameter("bi", [1, COUT], F32, isOutput=False)
    Wo = nc.declare_dram_parameter("Wo", [COUT, NCLS], F32, isOutput=False)
    bo = nc.declare_dram_parameter("bo", [1, NCLS], F32, isOutput=False)
    dl = nc.declare_dram_parameter("dl", [1, 1], F32, isOutput=False)
    av = nc.declare_dram_parameter("av", [1, 1], F32, isOutput=False)
    idt = nc.declare_dram_parameter("idt", [PART, PART], BF16, isOutput=False)
    sbl = nc.declare_dram_parameter("sblob", [PART, g.nchunks, W], BF16,
                                    isOutput=False)
    ibl = nc.declare_dram_parameter("iblob", [16, g.idx_cols_total], I16,
                                    isOutput=False)
    out = nc.declare_dram_parameter("out", [SHARD, NCLS], F32, isOutput=True)

    # ---------------- internal dram
    agin = [nc.dram_tensor(f"agin{k}", [SHARD, COUT], BF16) for k in range(4)]
    tbl = [nc.dram_tensor(f"tbl{k}", [N, COUT], BF16, addr_space="Shared")
           for k in range(4)]
    mm_in = nc.dram_tensor("mm_in", [1, 2], F32)
    mm_out = nc.dram_tensor("mm_out", [1, 2], F32, addr_space="Shared")

    def store_piece(k, sb3, pi):
        """Store piece pi's blocks of sb3 [128, NT, COUT] into agin[k]."""
        b0, b1, off, rows = cfg.pieces[pi]
        full = b1 - b0 - (1 if b1 == NT else 0)
        if full:
            nc.sync.dma_start(
                agin[k][off:off + full * PART, :].rearrange(
                    "(t p) c -> p t c", p=PART),
                sb3[:, b0:b0 + full, :])
        if b1 == NT:
            nc.sync.dma_start(agin[k][off + full * PART:off + rows, :],
                              sb3[0:LT, NT - 1, :])

    def ag_piece(k, pi):
        """AllGather piece pi of agin[k] into the piece-major tbl[k]."""
        _, _, off, rows = cfg.pieces[pi]
        t0 = int(cfg.poff[pi])
        nc.gpsimd.collective_compute(
            "AllGather", ALU.bypass, RG,
            ins=[agin[k][off:off + rows, :]],
            outs=[tbl[k][t0:t0 + PCORES * rows, :]])

    def transpose(out_ap, in_ap, ident, k):
        nc.tensor.matmul(out_ap, in_ap, ident[0:k, 0:k], is_transpose=True,
                         start=True, stop=True, skip_group_check=True)

    with tile.TileContext(nc) as tc:
        with (
            tc.tile_pool(name="const", bufs=1) as cp,
            tc.tile_pool(name="resid", bufs=1) as rp,
            tc.tile_pool(name="gat", bufs=2 * NQ + 2) as gp,
            tc.tile_pool(name="stage", bufs=3) as sp,
            tc.tile_pool(name="pbig", bufs=4, space="PSUM") as pb,
            tc.tile_pool(name="psmall", bufs=2, space="PSUM") as ps,
            tc.tile_pool(name="phd", bufs=2, space="PSUM") as ph,
        ):
            # ---------------- constants
            Wi_sb = cp.tile([PART, cfg.CINT, COUT], BF16)
            for h in range(cfg.CINT):
                hi = min(cfg.CIN - h * PART, PART)
                nc.gpsimd.dma_start(Wi_sb[0:hi, h, :],
                                    Wi[h * PART:h * PART + hi, :])
            bi_sb = cp.tile([1, COUT], BF16)
            nc.gpsimd.dma_start(bi_sb[:], bi[:])
            Wo_sb = cp.tile([PART, NCLS], BF16)
            nc.gpsimd.dma_start(Wo_sb[0:COUT, :], Wo[:])
            bo_sb = cp.tile([1, NCLS], BF16)
            nc.gpsimd.dma_start(bo_sb[:], bo[:])
            id_bf = cp.tile([PART, PART], BF16)
            nc.sync.dma_start(id_bf[:], idt[:])
            id_f32 = cp.tile([PART, PART], F32)
            nc.vector.tensor_copy(id_f32[:], id_bf[:])
            ones1b = cp.tile([1, PART], BF16)
            nc.vector.memset(ones1b[:], 1.0)
            ones1f = cp.tile([1, PART], F32)
            nc.vector.memset(ones1f[:], 1.0)
            onesCb = cp.tile([PART, COUT], BF16)
            nc.vector.memset(onesCb[:], 1.0)
            dl_sb = cp.tile([1, 1], F32)
            nc.sync.dma_start(dl_sb[:], dl[:])
            av_sb = cp.tile([1, 1], F32)
            nc.sync.dma_start(av_sb[:], av[:])

            idx_sb = cp.tile([PART, g.idx_cols_total], I16)
            for k in range(8):
                nc.sync.dma_start(idx_sb[16 * k:16 * (k + 1), :], ibl[:])
            s_sb = cp.tile([PART, g.nchunks, W], BF16)
            nc.sync.dma_start(s_sb[:], sbl[:])

            # resident feature tiles [128, NT, COUT]
            h_sb = rp.tile([PART, NT, COUT], BF16)
            t1_sb = rp.tile([PART, NT, COUT], BF16)
            t2_sb = rp.tile([PART, NT, COUT], BF16)
            u_sb = rp.tile([PART, NT, COUT], BF16)
            dn_sb = rp.tile([PART, NT], F32)
            nc.vector.memset(dn_sb[:], 0.0)
            hd_sb = rp.tile([PART, NT], F32)
            ns_sb = rp.tile([PART, NT], F32)

            # ---------------- coefficients (f32 [1,1] lane ops)
            cofp = cp.tile([1, 8], F32)  # c2L c1L c0L1 n2d d2
            nc.vector.memset(cofp[:], 0.0)
            t_ = cp.tile([1, 6], F32)
            d_, a_ = dl_sb[:], av_sb[:]
            d2_ = t_[:, 0:1]
            nc.vector.tensor_mul(d2_, d_, d_)
            d3_ = t_[:, 1:2]
            nc.vector.tensor_mul(d3_, d2_, d_)
            da_ = t_[:, 2:3]
            nc.vector.tensor_mul(da_, d_, a_)
            d2a_ = t_[:, 3:4]
            nc.vector.tensor_mul(d2a_, d2_, a_)
            nc.vector.scalar_tensor_tensor(cofp[:, 0:1], d_, -3.0, a_,
                                           ALU.mult, ALU.subtract)  # -3d - a
            t30 = t_[:, 4:5]
            nc.vector.tensor_scalar_mul(t30, d2_, 3.0)
            nc.vector.scalar_tensor_tensor(cofp[:, 1:2], da_, 2.0, t30,
                                           ALU.mult, ALU.add)  # 3d2 + 2da
            t31 = t_[:, 5:6]
            nc.vector.tensor_add(t31, d3_, d2a_)
            nc.vector.tensor_scalar(cofp[:, 2:3], t31, -1.0, -1.0,
                                    ALU.mult, ALU.add)  # -(d3 + d2a) - 1
            nc.vector.tensor_scalar_mul(cofp[:, 3:4], d_, -2.0)  # -2d
            nc.vector.tensor_copy(cofp[:, 4:5], d2_)  # d2
            pco = ps.tile([PART, 8], F32, tag="sm")
            nc.tensor.matmul(pco[:], ones1f[:], cofp[:], start=True, stop=True)
            cf_sb = cp.tile([PART, 8], F32)
            nc.scalar.copy(cf_sb[:], pco[:])
            C2L, C1L, C0L1 = cf_sb[:, 0:1], cf_sb[:, 1:2], cf_sb[:, 2:3]
            N2D, D2C = cf_sb[:, 3:4], cf_sb[:, 4:5]

            # ---------------- prologue: h = tanh(x @ Wi + bi)
            with tc.tile_pool(name="xt", bufs=3) as xp:
                for t in range(NT):
                    vp = LT if t == NT - 1 else PART
                    r0 = t * PART
                    xTt = xp.tile([PART, cfg.CINT, PART], BF16,
                                  name="xTt", tag="xTt")
                    for h in range(cfg.CINT):
                        hi = min(cfg.CIN - h * PART, PART)
                        nc.sync.dma_start(
                            xTt[0:hi, h, 0:vp],
                            xs[h * PART:h * PART + hi, r0:r0 + vp])
                    pt = pb.tile([PART, COUT], F32, tag="big")
                    for h in range(cfg.CINT):
                        nc.tensor.matmul(pt[0:vp, :], xTt[:, h, 0:vp],
                                         Wi_sb[:, h, :], start=(h == 0),
                                         stop=False)
                    nc.tensor.matmul(pt[0:vp, :], ones1b[:, 0:vp], bi_sb[:],
                                     start=False, stop=True)
                    nc.scalar.activation(h_sb[0:vp, t, :], pt[0:vp, :],
                                         AF.Tanh)
                    if t in cfg.piece_end:
                        pi = cfg.piece_end[t]
                        store_piece(0, h_sb, pi)
                        ag_piece(0, pi)

            def _early_out():
                for t in range(NT):
                    vp = LT if t == NT - 1 else PART
                    zt = sp.tile([PART, NCLS], F32, tag="es", name="zt")
                    nc.vector.memset(zt[:], 0.0)
                    nc.gpsimd.dma_start(out[t * PART:t * PART + vp, :],
                                        zt[0:vp, :])

            # ---------------- generic sparse hop
            def hop(tbl_in, nfree, on_block_done):
                tA = tbl_in[0:cfg.TA_H, :]
                tB = tbl_in[cfg.B_BASE:N, :]
                gq = [0]
                for bt in g.batches:
                    gA = gB = None
                    if bt["ncA"]:
                        gA = gp.tile([PART, bt["ncA"], COUT], BF16, tag="g")
                        nc.gpsimd.dma_gather(
                            gA[:], tA,
                            idx_sb[:, bt["idx_colA"]:
                                   bt["idx_colA"] + bt["ncA"] * 8],
                            bt["ncA"] * PART, bt["ncA"] * PART, COUT,
                            single_packet=False, queue_num=gq[0] % NQ)
                        gq[0] += 1
                    if bt["ncB"]:
                        gB = gp.tile([PART, bt["ncB"], COUT], BF16, tag="g")
                        nc.gpsimd.dma_gather(
                            gB[:], tB,
                            idx_sb[:, bt["idx_colB"]:
                                   bt["idx_colB"] + bt["ncB"] * 8],
                            bt["ncB"] * PART, bt["ncB"] * PART, COUT,
                            single_packet=False, queue_num=gq[0] % NQ)
                        gq[0] += 1
                    cur_b = -1
                    psum = None
                    for c in bt["chunks"]:
                        if c["b"] != cur_b:
                            if cur_b >= 0:
                                on_block_done(cur_b, psum)
                            cur_b = c["b"]
                            if nfree == COUT:
                                psum = pb.tile([PART, COUT], F32, tag="big",
                                               name="psum_hop")
                            else:
                                psum = ph.tile([PART, 1], F32, tag="hd",
                                               name="psum_hd")
                        woff = (c["w"] % cfg.WPB) * W
                        gt = gA if c["tab"] == 0 else gB
                        nc.tensor.matmul(
                            psum[woff:woff + W, 0:nfree],
                            s_sb[:, c["g"], :],
                            gt[:, c["pos"], 0:nfree],
                            start=c["first"], stop=c["last"],
                            skip_group_check=True,
                            tile_position=(0, woff))
                    if cur_b >= 0:
                        on_block_done(cur_b, psum)

            if STAGE < 1:
                _early_out()
                _finalize(nc)
                return nc

            # hop 1: T1
            def done1(b, psum):
                vp = LT if b == NT - 1 else PART
                nc.scalar.copy(t1_sb[0:vp, b, :], psum[0:vp, :])
                if b in cfg.piece_end:
                    pi = cfg.piece_end[b]
                    store_piece(1, t1_sb, pi)
                    ag_piece(1, pi)
            hop(tbl[0], COUT, done1)

            if STAGE < 2:
                _early_out()
                _finalize(nc)
                return nc

            # hop 2: T2 (db computed later, in hop 3's callback)
            def done2(b, psum):
                vp = LT if b == NT - 1 else PART
                nc.scalar.copy(t2_sb[0:vp, b, :], psum[0:vp, :])
                if b in cfg.piece_end:
                    pi = cfg.piece_end[b]
                    store_piece(2, t2_sb, pi)
                    ag_piece(2, pi)
            hop(tbl[1], COUT, done2)

            if STAGE < 3:
                _early_out()
                _finalize(nc)
                return nc

            # hop 3: u = T3 + c2L*T2 + c1L*T1 + (c0L-1)*h ; dn2 = row sumsq
            def done3(b, psum):
                vp = LT if b == NT - 1 else PART
                e1 = sp.tile([PART, COUT], F32, tag="e1")
                nc.vector.scalar_tensor_tensor(
                    e1[0:vp, :], t2_sb[0:vp, b, :], C2L[0:vp, :],
                    psum[0:vp, :], ALU.mult, ALU.add)
                nc.vector.scalar_tensor_tensor(
                    e1[0:vp, :], t1_sb[0:vp, b, :], C1L[0:vp, :],
                    e1[0:vp, :], ALU.mult, ALU.add)
                nc.vector.scalar_tensor_tensor(
                    u_sb[0:vp, b, :], h_sb[0:vp, b, :], C0L1[0:vp, :],
                    e1[0:vp, :], ALU.mult, ALU.add)
                sq = sp.tile([PART, COUT], F32, tag="sq")
                nc.scalar.activation(sq[0:vp, :], u_sb[0:vp, b, :], AF.Square,
                                     accum_out=dn_sb[0:vp, b:b + 1])
                # t2 is no longer needed as-is: overwrite with
                # db = T2 - 2d*T1 + d2*h for the epilogue
                nc.vector.scalar_tensor_tensor(
                    t2_sb[0:vp, b, :], t1_sb[0:vp, b, :], N2D[0:vp, :],
                    t2_sb[0:vp, b, :], ALU.mult, ALU.add)
                nc.vector.scalar_tensor_tensor(
                    t2_sb[0:vp, b, :], h_sb[0:vp, b, :], D2C[0:vp, :],
                    t2_sb[0:vp, b, :], ALU.mult, ALU.add)
                if b in cfg.piece_end:
                    # dn = sqrt(sumsq) for this piece's blocks, broadcast
                    # across channels into agin[3], AllGather into tbl[3]
                    pi = cfg.piece_end[b]
                    b0, b1, off, rows = cfg.pieces[pi]
                    nc.scalar.sqrt(dn_sb[:, b0:b1], dn_sb[:, b0:b1])
                    for bb in range(b0, b1):
                        vpb = LT if bb == NT - 1 else PART
                        dnb_t = sp.tile([PART, COUT], BF16, tag="fr")
                        nc.vector.tensor_scalar_mul(dnb_t[0:vpb, :],
                                                    onesCb[0:vpb, :],
                                                    dn_sb[0:vpb, bb:bb + 1])
                        nc.sync.dma_start(
                            agin[3][bb * PART:bb * PART + vpb, :],
                            dnb_t[0:vpb, :])
                    ag_piece(3, pi)
            hop(tbl[2], COUT, done3)

            if STAGE < 4:
                _early_out()
                _finalize(nc)
                return nc

            if STAGE < 5:
                _early_out()
                _finalize(nc)
                return nc

            # hop 4: hd = L @ dn (vector is idle here: fold u <- u + h
            # so the epilogue's final mix is a single op per tile)
            def done4(b, psum):
                vp = LT if b == NT - 1 else PART
                nc.scalar.copy(hd_sb[0:vp, b:b + 1], psum[0:vp, 0:1])
                nc.vector.tensor_add(u_sb[0:vp, b, :], u_sb[0:vp, b, :],
                                     h_sb[0:vp, b, :])
            hop(tbl[3], 1, done4)

            if STAGE < 6:
                _early_out()
                _finalize(nc)
                return nc

            # ---------------- global min/max -> AllReduce(max) of [max, -min]
            mx1 = sp.tile([PART, 1], F32, tag="mm")
            mn1 = sp.tile([PART, 1], F32, tag="mm")
            if NT > 1:
                nc.vector.reduce_max(mx1[:], hd_sb[:, 0:NT - 1], axis=AXL.X)
                nc.vector.tensor_reduce(mn1[:], hd_sb[:, 0:NT - 1], axis=AXL.X, op=ALU.min)
                nc.vector.tensor_tensor(mx1[0:LT, :], mx1[0:LT, :],
                                        hd_sb[0:LT, NT - 1:NT], op=ALU.max)
                nc.vector.tensor_tensor(mn1[0:LT, :], mn1[0:LT, :],
                                        hd_sb[0:LT, NT - 1:NT], op=ALU.min)
            else:
                nc.vector.reduce_max(mx1[0:LT, :], hd_sb[0:LT, :], axis=AXL.X)
                nc.vector.tensor_reduce(mn1[0:LT, :], hd_sb[0:LT, :],
                                        axis=AXL.X, op=ALU.min)
            nc.vector.tensor_scalar_mul(mn1[:], mn1[:], -1.0)
            pmx = ps.tile([PART, PART], F32, tag="sm")
            transpose(pmx[0:1, 0:PART], mx1[:], id_f32, PART)
            pmn = ps.tile([PART, PART], F32, tag="sm")
            transpose(pmn[0:1, 0:PART], mn1[:], id_f32, PART)
            mm_sb = sp.tile([1, 2], F32, tag="mm2")
            nc.vector.reduce_max(mm_sb[0:1, 0:1], pmx[0:1, 0:PART], axis=AXL.X)
            nc.vector.reduce_max(mm_sb[0:1, 1:2], pmn[0:1, 0:PART], axis=AXL.X)
            nc.sync.dma_start(mm_in[:], mm_sb[:])
            nc.gpsimd.collective_compute("AllReduce", ALU.max, RG,
                                         ins=[mm_in[:]], outs=[mm_out[:]])
            mmg = sp.tile([1, 2], F32, tag="mm2")
            nc.sync.dma_start(mmg[:], mm_out[:])

            # s = 2a/(mx - mn); ns = (hd + (-mn)) * s  (= normal * 2a)
            sc = cp.tile([1, 2], F32)
            nc.vector.tensor_add(sc[:, 0:1], mmg[:, 0:1], mmg[:, 1:2])
            nc.vector.reciprocal(sc[:, 0:1], sc[:, 0:1])
            nc.vector.tensor_mul(sc[:, 0:1], sc[:, 0:1], av_sb[:])
            nc.vector.tensor_scalar_mul(sc[:, 0:1], sc[:, 0:1], 2.0)
            nc.vector.tensor_copy(sc[:, 1:2], mmg[:, 1:2])
            pbc = ps.tile([PART, PART], F32, tag="sm")
            nc.tensor.matmul(pbc[0:PART, 0:2], ones1f[:], sc[:],
                             start=True, stop=True, skip_group_check=True)
            bc = cp.tile([PART, 2], F32)
            nc.scalar.copy(bc[:], pbc[0:PART, 0:2])
            nc.vector.tensor_scalar(ns_sb[:], hd_sb[:], bc[:, 1:2],
                                    bc[:, 0:1], ALU.add, ALU.mult)

            if STAGE < 7:
                _early_out()
                _finalize(nc)
                return nc

            # ---------------- epilogue (phase-split to avoid act-table thrash)
            pys = rp.tile([PART, NT, NCLS], F32)
            nm_sb = rp.tile([PART, NT], F32)
            ss_sb = rp.tile([PART, NT], F32)
            lse_sb = rp.tile([PART, NT], F32)
            nc.vector.memset(ss_sb[:], 1.0)
            for t in range(NT):
                vp = LT if t == NT - 1 else PART
                # t2_sb holds db, u_sb holds u + h (precomputed in hops 3/4)
                f1 = sp.tile([PART, COUT], F32, tag="f1")
                nc.vector.scalar_tensor_tensor(
                    f1[0:vp, :], t2_sb[0:vp, t, :], ns_sb[0:vp, t:t + 1],
                    u_sb[0:vp, t, :], ALU.mult, ALU.add)
                fr = sp.tile([PART, COUT], BF16, tag="fr")
                nc.scalar.activation(fr[0:vp, :], f1[0:vp, :], AF.Relu)
                ptr = pb.tile([PART, PART], BF16, tag="big")
                transpose(ptr[0:COUT, 0:vp], fr[0:vp, :], id_bf, vp)
                frT = sp.tile([PART, PART], BF16, tag="frT")
                nc.vector.tensor_copy(frT[0:COUT, 0:vp], ptr[0:COUT, 0:vp])
                py = ps.tile([PART, NCLS], F32, tag="sm")
                nc.tensor.matmul(py[0:vp, :], frT[:, 0:vp], Wo_sb[:],
                                 start=True, stop=False)
                nc.tensor.matmul(py[0:vp, :], ones1b[:, 0:vp], bo_sb[:],
                                 start=False, stop=True)
                nc.vector.tensor_copy(pys[0:vp, t, :], py[0:vp, :])
                nc.vector.reduce_max(nm_sb[0:vp, t:t + 1], py[0:vp, :],
                                     axis=AXL.X, negate=True)
            for t in range(NT):
                vp = LT if t == NT - 1 else PART
                es = sp.tile([PART, NCLS], F32, tag="es")
                nc.scalar.activation(es[0:vp, :], pys[0:vp, t, :], AF.Exp,
                                     bias=nm_sb[0:vp, t:t + 1],
                                     accum_out=ss_sb[0:vp, t:t + 1])
            nc.scalar.activation(lse_sb[:], ss_sb[:], AF.Ln)
            for t in range(NT):
                vp = LT if t == NT - 1 else PART
                ot = sp.tile([PART, NCLS], F32, tag="es")
                nc.vector.tensor_scalar(ot[0:vp, :], pys[0:vp, t, :],
                                        nm_sb[0:vp, t:t + 1],
                                        lse_sb[0:vp, t:t + 1],
                                        ALU.add, ALU.subtract)
                r0 = t * PART
                nc.sync.dma_start(out[r0:r0 + vp, :], ot[0:vp, :])

    _finalize(nc)
    return nc


# ---------------------------------------------------------------------------
# entry point
# ---------------------------------------------------------------------------

def _in_maps(cfg, g, x, W_in, b_in, W_out, b_out, delta, a):
    ident = np.eye(PART, dtype=np.float32).astype(ml_dtypes.bfloat16)
    maps = []
    for m in range(PCORES):
        maps.append({
            "xs": np.ascontiguousarray(
                x[m * cfg.SHARD:(m + 1) * cfg.SHARD].T).astype(
                    ml_dtypes.bfloat16),
            "Wi": np.ascontiguousarray(W_in).astype(np.float32),
            "bi": b_in.reshape(1, -1).astype(np.float32),
            "Wo": np.ascontiguousarray(W_out).astype(np.float32),
            "bo": b_out.reshape(1, -1).astype(np.float32),
            "dl": delta.reshape(1, 1).astype(np.float32),
            "av": a.reshape(1, 1).astype(np.float32),
            "idt": ident,
            "sblob": g.sblobs[m],
            "iblob": g.iblobs[m],
        })
    return maps


def prepare(x, vals, W_in, b_in, delta, a, W_out, b_out, rows, cols,
            debug=False, **cfg_kw):
    x = np.asarray(x)
    cfg = Cfg(N=x.shape[0], E=len(np.asarray(vals)), CIN=x.shape[1],
              COUT=np.asarray(W_in).shape[1], NCLS=np.asarray(W_out).shape[1],
              **cfg_kw)
    g = build_grid(cfg, np.asarray(rows), np.asarray(cols),
                   np.asarray(vals, np.float32))
    nc = build_nc(cfg, g, debug=debug)
    maps = _in_maps(cfg, g, x, np.asarray(W_in), np.asarray(b_in),
                    np.asarray(W_out), np.asarray(b_out),
                    np.asarray(delta), np.asarray(a))
    return cfg, g, nc, maps


def kernel(x, vals, W_in, b_in, delta, a, W_out, b_out, rows, cols):
    from concourse.bass_utils import run_bass_kernel_spmd

    cfg, g, nc, maps = prepare(x, vals, W_in, b_in, delta, a, W_out, b_out,
                               rows, cols)
    res = run_bass_kernel_spmd(nc, maps, core_ids=list(range(PCORES)))
    return np.concatenate([res.results[m]["out"] for m in range(PCORES)], 0)

